# revision 20
# baseline (speedup 1.0000x reference)
"""Trainium2 Bass kernel for the ODEFunc GNN message-passing module.

Math (B=2, N=512, H=128, O=32):
    q = z @ Wq.T + bq ;  k = s_t @ Wk.T + bk
    scores = (q @ k.T)/sqrt(H), diagonal masked to -inf
    attn = softmax_j(scores)
    U    = sum_j attn[i,j] * tanh(xi_i + yj_j)      (xi = z@W1i.T + b1, yj = z@W1j.T)
    agg  = U @ W2.T + b2     (softmax rows sum to 1 -> W2 moves after aggregation)
    dz   = tanh(agg @ W3.T + b3) @ W4.T + b4

Key trick: expand tanh in a factorized basis
    tanh(x) ~ LIN_C*x + sum_m AM[m]*sin(m*W*x)        on |x| <= 4.35
so with sin(m w (xi+yj)) = sin(m w xi)cos(m w yj) + cos(m w xi)sin(m w yj),
the attention aggregation becomes moment matmuls E^T @ [1 | z | sin | cos]
with E[j,i] = exp(scores) (unnormalized, diag-zeroed).  The xi-linear and
z-moment-linear terms fold into extra epilogue matmuls; W3 is folded into
the W2-stage matrices (W2?3 = W2? @ W3T) so the epilogue is two matmul
stages; 1/ssum folds into the combine via the ones-column moment.

exp(s) = (1+tanh(s/2))/(1-tanh(s/2)) so sin+tanh suffice -> a single
manually-placed LoadActFuncSet(silu_and_others) covers every activation.
q/k projections fold into one [H,O] matrix (bk cancels in softmax).
On-chip derivations minimize input DMA: diag mask via iota+compare, the
m-scaled weight blocks via DVE scalar muls, F's z-columns via PE
transposes of zT.  All matmul operands fp16; fp32 PSUM accumulation.

Sharding: 1024 (b,i) pairs over 8 cores (batch-major, 128 i's per core).
"""

import numpy as np

B, N, H, O = 2, 512, 128, 32
NC = 8
CPB = NC // B  # cores per batch = 4
IPC = N // CPB  # i's per core = 128
NCH = N // 128  # j chunks = 4

# tanh(x) ~ LIN_C*x + sum_m AM[m] sin((m+1) W x), minimax fit on [-4.35, 4.35]
W = 0.9130
LIN_C = 0.289778
AM = [0.463016, 0.103367, 0.026572]
M = 3
MH = M * H  # 384
NF = H + 2 * MH  # 896 feature cols: [sin | z | cos]
HALF_PI = 1.5707963267948966
SILU_SET_ID = 18  # silu_and_others: contains both sin and tanh

# bigA packed columns (fp16, [128, .]) -- scores path
A_ZTI = 0             # zTi   [H, 128]
A_QK = 128            # QKmat [H, 32]
A_ID = 160            # identity [128, 128]
BIGA = 288
# bigB packed columns -- feature path (DMA'd first: heads the critical chain)
B_ZT = 0              # zT    [H, N]
B_W1J = N             # W1jT  [H, H]
B_W1I = N + 128       # W1iT  [H, H]
BIGB = N + 256
# F feature column layout: [sin | z | cos]; ssum via its own tiny moment
F_SIN = 0
F_Z = MH
F_COS = MH + H
# bigC packed columns -- epilogue weights
C_W2T3 = 0            # W2T@W3T [H, H]
C_W2J3 = 128          # W2J@W3T
C_W2I3 = 256          # W2I@W3T
C_W4T = 384           # W4T
BIGC = 512
# rows packed (fp16, [1, .])
R_ONES = 0
R_BIT = 128           # brow_it [1, MH]
R_BQK = 128 + MH      # bqk [1, 32]
ROWS = 160 + MH

_CACHE = {}

# Stash of the last BassKernelResults (exec_time_ns etc.) for test harnesses.
LAST_RESULTS = None


def _build():
    from contextlib import ExitStack

    import concourse.tile as tile
    from concourse import bacc, mybir

    f32 = mybir.dt.float32
    f16 = mybir.dt.float16
    i32 = mybir.dt.int32
    AF = mybir.ActivationFunctionType
    ALU = mybir.AluOpType

    nc = bacc.Bacc(trn_type="TRN2")

    bigA = nc.dram_tensor("bigA", [128, BIGA], f16, kind="ExternalInput")
    sT = nc.dram_tensor("sT", [O, N], f16, kind="ExternalInput")
    rows = nc.dram_tensor("rows", [1, ROWS], f16, kind="ExternalInput")
    bigB = nc.dram_tensor("bigB", [128, BIGB], f16, kind="ExternalInput")
    bigC = nc.dram_tensor("bigC", [128, BIGC], f16, kind="ExternalInput")
    bcols = nc.dram_tensor("bcols", [H, 4], f32, kind="ExternalInput")
    out = nc.dram_tensor("out", [H, IPC], f16, kind="ExternalOutput")

    with tile.TileContext(nc) as tc, ExitStack() as ctx:
        const = ctx.enter_context(tc.tile_pool(name="const", bufs=1))
        work = ctx.enter_context(tc.tile_pool(name="work", bufs=1))
        fpool = ctx.enter_context(tc.tile_pool(name="fpool", bufs=4))
        ps512 = ctx.enter_context(tc.tile_pool(name="ps512", bufs=2, space="PSUM"))
        psB = ctx.enter_context(tc.tile_pool(name="psB", bufs=2, space="PSUM"))
        psA = ctx.enter_context(tc.tile_pool(name="psA", bufs=1, space="PSUM"))
        psq = ctx.enter_context(tc.tile_pool(name="psq", bufs=1, space="PSUM"))
        psmom = ctx.enter_context(tc.tile_pool(name="psmom", bufs=1, space="PSUM"))

        # single activation-table load (sin + tanh live in silu_and_others)
        ld = mybir.InstLoadActFuncSet(
            act_func_set_id=SILU_SET_ID,
            name=nc.get_next_instruction_name(),
            engine=mybir.EngineType.Activation,
            ins=[],
            outs=[],
        )
        nc.scalar.add_instruction(ld)

        bigB_t = const.tile([128, BIGB], f16, tag="bigB", name="bigB_sb")
        nc.sync.dma_start(bigB_t[:], bigB[:, :])
        bigA_t = const.tile([128, BIGA], f16, tag="bigA", name="bigA_sb")
        nc.sync.dma_start(bigA_t[:], bigA[:, :])
        sT_t = const.tile([O, N], f16, tag="sT", name="sT_sb")
        nc.sync.dma_start(sT_t[:], sT[:, :])
        rows_t = const.tile([1, ROWS], f16, tag="rows", name="rows_sb")
        nc.sync.dma_start(rows_t[:], rows[:, :])
        bcols_t = const.tile([H, 4], f32, tag="bcols", name="bcols_sb")
        nc.sync.dma_start(bcols_t[:], bcols[:, :])
        bigC_t = const.tile([128, BIGC], f16, tag="bigC", name="bigC_sb")
        nc.sync.dma_start(bigC_t[:], bigC[:, :])

        zTi_s = bigA_t[:, A_ZTI : A_ZTI + 128]
        QK_s = bigA_t[:, A_QK : A_QK + 32]
        W1iT_s = bigB_t[:, B_W1I : B_W1I + 128]
        id_s = bigA_t[:, A_ID : A_ID + 128]
        zT_s = bigB_t[:, B_ZT : B_ZT + N]
        W1jT_s = bigB_t[:, B_W1J : B_W1J + 128]
        W2T3_s = bigC_t[:, C_W2T3 : C_W2T3 + 128]
        W2J3_s = bigC_t[:, C_W2J3 : C_W2J3 + 128]
        W2I3_s = bigC_t[:, C_W2I3 : C_W2I3 + 128]
        W4T_s = bigC_t[:, C_W4T : C_W4T + 128]
        ones_s = rows_t[:, R_ONES : R_ONES + 128]
        bit_s = rows_t[:, R_BIT : R_BIT + MH]
        bqk_s = rows_t[:, R_BQK : R_BQK + 32]
        b23_s = bcols_t[:, 0:1]
        b4_s = bcols_t[:, 1:2]
        blkv_s = bcols_t[:, 2:3]  # blk*128, per core

        hpi = work.tile([128, 1], f32, tag="hpi", name="hpi")
        nc.vector.memset(hpi[:], HALF_PI)

        # qkT[o, i] = (Wq.T@Wk/2sqrtH).T @ z_i.T + bqk  -> scores/2 = sT.T @ qkT
        qk_ps = psq.tile([32, 256], f16, tag="qk", name="qk_ps")
        qk32 = qk_ps[:, 0:256].bitcast(f32)
        nc.tensor.matmul(qk32, QK_s, zTi_s, start=True, stop=False)
        nc.tensor.matmul(qk32, bqk_s, ones_s, start=False, stop=True)
        qkT_t = work.tile([32, 128], f16, tag="qkT", name="qkT_sb")
        nc.vector.tensor_copy(qkT_t[:], qk32)

        # on-chip derived weights: RWj = [m w W1jT]_m, rit = [m w W1iT]_m
        RWj_t = work.tile([128, MH], f16, tag="RWj", name="RWj_sb")
        for m in range(M):
            nc.vector.tensor_scalar_mul(
                RWj_t[:, m * H : (m + 1) * H], W1jT_s, float((m + 1) * W)
            )
        rit_t = work.tile([128, MH], f16, tag="rit", name="rit_sb")
        for m in range(M):
            nc.vector.tensor_scalar_mul(
                rit_t[:, m * H : (m + 1) * H], W1iT_s, float((m + 1) * W)
            )
        idam_t = work.tile([128, M * 128], f16, tag="idam", name="idam_sb")
        for m in range(M):
            nc.vector.tensor_scalar_mul(
                idam_t[:, m * 128 : (m + 1) * 128], id_s, float(AM[m])
            )

        # F feature tiles [1 | sin | z | cos]; ones by memset, z by PE transpose
        F_t = []
        for c in range(NCH):
            fc = fpool.tile([128, NF], f16, tag="F", name=f"F{c}")
            F_t.append(fc)
        onec_t = work.tile([128, 1], f16, tag="onec", name="onec")
        nc.vector.memset(onec_t[:], 1.0)

        # iota d[p, col] = col - p  (for the diag mask compare)
        d_t = work.tile([128, N], i32, tag="d", name="d_sb")
        nc.gpsimd.iota(d_t[:], [[1, N]], base=0, channel_multiplier=-1)

        # scT[jj, c*128+i] = scores(i, j=c*128+jj)/2
        scT_ps = ps512.tile([128, N], f32, tag="b512", name="scT_ps")
        for c in range(NCH):
            nc.tensor.matmul(
                scT_ps[:, c * 128 : (c + 1) * 128],
                sT_t[:, c * 128 : (c + 1) * 128],
                qkT_t[:],
                start=True,
                stop=True,
            )
        th_t = work.tile([128, N], f32, tag="th", name="th_sb")
        nc.scalar.activation(th_t[:, 0:128], scT_ps[:, 0:128], AF.Tanh)
        nc.scalar.activation(th_t[:, 128:N], scT_ps[:, 128:N], AF.Tanh)

        # xi-side trig args: [m w xi]_m  (xi = z_i@W1iT + b1)
        xit_ps = ps512.tile([128, MH], f32, tag="b512", name="xit_ps")
        nc.tensor.matmul(xit_ps[:], zTi_s, rit_t[:], start=True, stop=False)
        nc.tensor.matmul(xit_ps[:], ones_s, bit_s, start=False, stop=True)
        xit_t = work.tile([128, MH], f32, tag="xit", name="xit_sb")
        nc.vector.tensor_copy(xit_t[:], xit_ps[:])

        # z-column blocks of F via PE transpose of zT chunks
        zt_ps = []
        for c in range(NCH):
            zp = psB.tile([128, 256], f16, tag="psB", name=f"zt{c}")
            nc.tensor.transpose(zp[:, 0:128], zT_s[:, c * 128 : (c + 1) * 128], id_s)
            zt_ps.append(zp)

        # E = exp(2*scT) = (1+th)/(1-th), diag zeroed; chunk 0 first so the
        # first moment matmul can start early.
        r1_t = work.tile([128, N], f32, tag="r1", name="r1")
        r2_t = work.tile([128, N], f32, tag="r2", name="r2")
        E_t = work.tile([128, N], f16, tag="E", name="E")
        for ci, (lo, hi) in enumerate(((0, 128), (128, N))):
            nc.vector.tensor_scalar(
                r1_t[:, lo:hi], th_t[:, lo:hi], -1.0, 1.0, ALU.mult, ALU.add
            )
            nc.vector.reciprocal_approx_fast(r2_t[:, lo:hi], r1_t[:, lo:hi])
            nc.vector.scalar_tensor_tensor(
                E_t[:, lo:hi], th_t[:, lo:hi], 1.0, r2_t[:, lo:hi],
                ALU.add, ALU.mult,
            )
            # E *= (d != blk*128): zeroes the attention diagonal
            nc.vector.scalar_tensor_tensor(
                E_t[:, lo:hi], d_t[:, lo:hi], blkv_s, E_t[:, lo:hi],
                ALU.not_equal, ALU.mult,
            )
            for c in ((0, 1) if ci == 0 else (2, 3)):
                nc.vector.tensor_copy(F_t[c][:, F_Z : F_Z + H], zt_ps[c][:, 0:128])

        # j-side features + moments, pipelined per chunk
        mom_ps = psmom.tile([128, 1024], f32, tag="mom", name="mom_ps")
        for c in range(NCH):
            nc.tensor.matmul(
                mom_ps[:, 896:897], E_t[:, c * 128 : (c + 1) * 128], onec_t[:],
                start=(c == 0), stop=(c == NCH - 1),
            )
        rs_t = work.tile([128, 1], f32, tag="rs", name="rs")
        nc.vector.reciprocal(rs_t[:], mom_ps[:, 896:897])
        slices = [(0, 512), (512, NF)]
        for c in range(NCH):
            xj_ps = ps512.tile([128, MH], f32, tag="b512", name=f"xj{c}")
            nc.tensor.matmul(
                xj_ps[:], zT_s[:, c * 128 : (c + 1) * 128], RWj_t[:],
                start=True, stop=True,
            )
            fc = F_t[c]
            nc.scalar.activation(fc[:, F_SIN : F_SIN + MH], xj_ps[:], AF.Sin)
            nc.scalar.activation(
                fc[:, F_COS : F_COS + MH], xj_ps[:], AF.Sin, bias=hpi[:, 0:1]
            )
            for s0, s1 in slices:
                nc.tensor.matmul(
                    mom_ps[:, s0:s1],
                    E_t[:, c * 128 : (c + 1) * 128],
                    fc[:, s0:s1],
                    start=(c == 0),
                    stop=(c == NCH - 1),
                )

        # xi-side trig (late in ACT queue: only needed by the combine)
        XiS = work.tile([128, MH], f16, tag="XiS", name="XiS")
        nc.scalar.activation(XiS[:], xit_t[:], AF.Sin)
        XiC = work.tile([128, MH], f16, tag="XiC", name="XiC")
        nc.scalar.activation(XiC[:], xit_t[:], AF.Sin, bias=hpi[:, 0:1])

        # combine: P = (XiS/ssum)*MCos + (XiC/ssum)*MSin; Tfin^T via
        # am-scaled-identity transpose-matmuls accumulating in PSUM.
        Mzn_t = work.tile([128, H], f16, tag="Mzn", name="Mzn")
        nc.scalar.activation(
            Mzn_t[:], mom_ps[:, F_Z : F_Z + H], AF.Identity, scale=rs_t[:, 0:1]
        )
        P2 = work.tile([128, MH], f16, tag="P2", name="P2")
        nc.vector.scalar_tensor_tensor(
            P2[:], XiC[:], rs_t[:, 0:1], mom_ps[:, F_SIN : F_SIN + MH],
            ALU.mult, ALU.mult,
        )
        P1 = work.tile([128, MH], f16, tag="P1", name="P1")
        nc.vector.scalar_tensor_tensor(
            P1[:], XiS[:], rs_t[:, 0:1], mom_ps[:, F_COS : F_COS + MH],
            ALU.mult, ALU.mult,
        )
        P = work.tile([128, MH], f16, tag="P", name="P")
        nc.vector.tensor_tensor(P[:], P1[:], P2[:], ALU.add)

        # epilogue, all in [h, i] layout; u = (W2@..@W3).T contributions
        u_ps = psA.tile([H, IPC], f32, tag="u", name="u_ps")
        nc.tensor.matmul(u_ps[:], W2I3_s, zTi_s, start=True, stop=False)
        mT_ps = psB.tile([128, 256], f16, tag="psB", name="mT_ps")
        nc.tensor.transpose(mT_ps[:, 0:128], Mzn_t[:], id_s)
        MzT = work.tile([128, IPC], f16, tag="MzT", name="MzT")
        nc.scalar.activation(MzT[:], mT_ps[:, 0:128], AF.Copy)
        PT_ps = psq.tile([128, 256], f16, tag="qk", name="PT_ps")
        PT32 = PT_ps[:, 0:256].bitcast(f32)
        for m in range(M):
            nc.tensor.matmul(
                PT32, P[:, m * H : (m + 1) * H],
                idam_t[:, m * 128 : (m + 1) * 128],
                start=(m == 0), stop=(m == M - 1),
            )
        acc_t = work.tile([128, IPC], f16, tag="acc", name="acc")
        nc.vector.tensor_copy(acc_t[:], PT32)
        nc.tensor.matmul(u_ps[:], W2J3_s, MzT[:], start=False, stop=False)
        nc.tensor.matmul(u_ps[:], W2T3_s, acc_t[:], start=False, stop=True)

        t3_t = work.tile([H, IPC], f16, tag="t3", name="t3_sb")
        nc.scalar.activation(t3_t[:], u_ps[:], AF.Tanh, bias=b23_s)
        dz_ps = psA.tile([H, IPC], f32, tag="u", name="dz_ps")
        nc.tensor.matmul(dz_ps[:], W4T_s, t3_t[:], start=True, stop=True)
        dzT = work.tile([H, IPC], f16, tag="dzT", name="dzT_sb")
        nc.scalar.activation(dzT[:], dz_ps[:], AF.Identity, bias=b4_s)
        for r in range(16):
            nc.sync.dma_start(out[8 * r : 8 * (r + 1), :], dzT[8 * r : 8 * (r + 1), :])

    nc.finalize()
    return nc


def _get_nc():
    if "nc" not in _CACHE:
        _CACHE["nc"] = _build()
    return _CACHE["nc"]


def kernel(**inputs):
    global LAST_RESULTS
    from concourse.bass_utils import run_bass_kernel_spmd

    f = np.float32
    z = np.asarray(inputs["z"], f)
    s_t = np.asarray(inputs["s_t"], f)
    W1 = np.asarray(inputs["W1"], f)
    b1 = np.asarray(inputs["b1"], f)
    W2 = np.asarray(inputs["W2"], f)
    b2 = np.asarray(inputs["b2"], f)
    Wq = np.asarray(inputs["Wq"], f)
    bq = np.asarray(inputs["bq"], f)
    Wk = np.asarray(inputs["Wk"], f)
    W3 = np.asarray(inputs["W3"], f)
    b3 = np.asarray(inputs["b3"], f)
    W4 = np.asarray(inputs["W4"], f)
    b4 = np.asarray(inputs["b4"], f)

    h16 = np.float16
    tr = lambda m: np.ascontiguousarray(m.T, f)

    rt = f(1.0 / (2.0 * np.sqrt(H)))
    W1iT = tr(W1[:, :H])
    W1jT = tr(W1[:, H:])
    W2T = tr(W2)
    W3T = tr(W3)
    QKmat = (Wq.T @ Wk) * rt
    bqk = (bq @ Wk) * rt
    brow_it = np.concatenate([(m + 1) * W * b1 for m in range(M)])
    W2T3 = W2T @ W3T
    W2J3 = (LIN_C * (W1jT @ W2T)) @ W3T
    W2I3 = (LIN_C * (W1iT @ W2T)) @ W3T
    b23 = (b2 + LIN_C * (b1 @ W2T)) @ W3T + b3

    rows = np.zeros((1, ROWS), h16)
    rows[0, R_ONES : R_ONES + 128] = 1.0
    rows[0, R_BIT : R_BIT + MH] = brow_it.astype(h16)
    rows[0, R_BQK : R_BQK + 32] = bqk.astype(h16)

    bigA_shared = np.zeros((128, BIGA), h16)
    bigA_shared[:, A_QK : A_QK + 32] = QKmat.astype(h16)
    bigA_shared[:, A_ID : A_ID + 128] = np.eye(128, dtype=h16)
    bigC = np.zeros((128, BIGC), h16)
    bigC[:, C_W2T3 : C_W2T3 + 128] = W2T3.astype(h16)
    bigC[:, C_W2J3 : C_W2J3 + 128] = W2J3.astype(h16)
    bigC[:, C_W2I3 : C_W2I3 + 128] = W2I3.astype(h16)
    bigC[:, C_W4T : C_W4T + 128] = tr(W4).astype(h16)

    in_maps = []
    for c in range(NC):
        b, blk = divmod(c, CPB)
        i0 = blk * IPC
        bigA = bigA_shared.copy()
        bigA[:, A_ZTI : A_ZTI + 128] = z[b, i0 : i0 + IPC].T.astype(h16)
        bigB = np.zeros((128, BIGB), h16)
        bigB[:, B_ZT : B_ZT + N] = z[b].T.astype(h16)
        bigB[:, B_W1J : B_W1J + 128] = W1jT.astype(h16)
        bigB[:, B_W1I : B_W1I + 128] = W1iT.astype(h16)
        bcols = np.zeros((H, 4), f)
        bcols[:, 0] = b23
        bcols[:, 1] = b4
        bcols[:, 2] = blk * 128
        in_maps.append(
            dict(
                bigA=bigA,
                sT=s_t[b].T.astype(h16),
                rows=rows,
                bigB=bigB,
                bigC=bigC,
                bcols=bcols,
            )
        )

    nc = _get_nc()
    res = run_bass_kernel_spmd(nc, in_maps, core_ids=list(range(NC)))
    LAST_RESULTS = res

    dz = np.empty((B, N, H), dtype=f)
    for c in range(NC):
        b, blk = divmod(c, CPB)
        i0 = blk * IPC
        dz[b, i0 : i0 + IPC, :] = res.results[c]["out"].T.astype(f)
    return dz


# revision 21
# speedup vs baseline: 1.1590x; 1.1590x over previous
"""Trainium2 Bass kernel for the ODEFunc GNN message-passing module.

Math (B=2, N=512, H=128, O=32):
    q = z @ Wq.T + bq ;  k = s_t @ Wk.T + bk
    scores = (q @ k.T)/sqrt(H), diagonal masked to -inf
    attn = softmax_j(scores)
    U    = sum_j attn[i,j] * tanh(xi_i + yj_j)      (xi = z@W1i.T + b1, yj = z@W1j.T)
    agg  = U @ W2.T + b2     (softmax rows sum to 1 -> W2 moves after aggregation)
    dz   = tanh(agg @ W3.T + b3) @ W4.T + b4

Key trick: expand tanh in a factorized basis
    tanh(x) ~ LIN_C*x + sum_m AM[m]*sin(m*W*x)        on |x| <= 4.35
so with sin(m w (xi+yj)) = sin(m w xi)cos(m w yj) + cos(m w xi)sin(m w yj),
the attention aggregation becomes moment matmuls E^T @ [1 | z | sin | cos]
with E[j,i] = exp(scores) (unnormalized, diag-zeroed).  The xi-linear and
z-moment-linear terms fold into extra epilogue matmuls; W3 is folded into
the W2-stage matrices (W2?3 = W2? @ W3T) so the epilogue is two matmul
stages; 1/ssum folds into the combine via the ones-column moment.

exp(s) = (1+tanh(s/2))/(1-tanh(s/2)) so sin+tanh suffice -> a single
manually-placed LoadActFuncSet(silu_and_others) covers every activation.
q/k projections fold into one [H,O] matrix (bk cancels in softmax).
On-chip derivations minimize input DMA: diag mask via iota+compare, the
m-scaled weight blocks via DVE scalar muls, F's z-columns via PE
transposes of zT.  All matmul operands fp16; fp32 PSUM accumulation.

Sharding: 1024 (b,i) pairs over 8 cores (batch-major, 128 i's per core).
"""

import numpy as np

B, N, H, O = 2, 512, 128, 32
NC = 8
CPB = NC // B  # cores per batch = 4
IPC = N // CPB  # i's per core = 128
NCH = N // 128  # j chunks = 4

# tanh(x) ~ LIN_C*x + sum_m AM[m] sin((m+1) W x), minimax fit on [-4.35, 4.35]
W = 0.9130
LIN_C = 0.289778
AM = [0.463016, 0.103367, 0.026572]
M = 3
MH = M * H  # 384
NF = H + 2 * MH  # 896 feature cols: [sin | z | cos]
HALF_PI = 1.5707963267948966
SILU_SET_ID = 18  # silu_and_others: contains both sin and tanh

# bigA packed columns (fp16, [128, .]) -- scores path
A_ZTI = 0             # zTi   [H, 128]
A_QK = 128            # QKmat [H, 32]
A_ID = 160            # identity [128, 128]
BIGA = 288
# bigB packed columns -- feature path (DMA'd first: heads the critical chain)
B_ZT = 0              # zT    [H, N]
B_W1J = N             # W1jT  [H, H]
B_W1I = N + 128       # W1iT  [H, H]
BIGB = N + 256
# F feature column layout: [sin | z | cos]; ssum via its own tiny moment
F_SIN = 0
F_Z = MH
F_COS = MH + H
# bigC packed columns -- epilogue weights
C_W2T3 = 0            # W2T@W3T [H, H]
C_W2J3 = 128          # W2J@W3T
C_W2I3 = 256          # W2I@W3T
C_W4T = 384           # W4T
BIGC = 512
# rows packed (fp16, [1, .])
R_ONES = 0
R_BIT = 128           # brow_it [1, MH]
R_BQK = 128 + MH      # bqk [1, 32]
ROWS = 160 + MH

_CACHE = {}

# Stash of the last BassKernelResults (exec_time_ns etc.) for test harnesses.
LAST_RESULTS = None


def _build():
    from contextlib import ExitStack

    import concourse.tile as tile
    from concourse import bacc, mybir

    f32 = mybir.dt.float32
    f16 = mybir.dt.float16
    i32 = mybir.dt.int32
    AF = mybir.ActivationFunctionType
    ALU = mybir.AluOpType

    nc = bacc.Bacc(trn_type="TRN2")

    bigA = nc.dram_tensor("bigA", [128, BIGA], f16, kind="ExternalInput")
    sT = nc.dram_tensor("sT", [O, N], f16, kind="ExternalInput")
    rows = nc.dram_tensor("rows", [1, ROWS], f16, kind="ExternalInput")
    bigB = nc.dram_tensor("bigB", [128, BIGB], f16, kind="ExternalInput")
    bigC = nc.dram_tensor("bigC", [128, BIGC], f16, kind="ExternalInput")
    bcols = nc.dram_tensor("bcols", [H, 4], f32, kind="ExternalInput")
    out = nc.dram_tensor("out", [H, IPC], f16, kind="ExternalOutput")

    with tile.TileContext(nc) as tc, ExitStack() as ctx:
        const = ctx.enter_context(tc.tile_pool(name="const", bufs=1))
        work = ctx.enter_context(tc.tile_pool(name="work", bufs=1))
        fpool = ctx.enter_context(tc.tile_pool(name="fpool", bufs=4))
        ps512 = ctx.enter_context(tc.tile_pool(name="ps512", bufs=2, space="PSUM"))
        psB = ctx.enter_context(tc.tile_pool(name="psB", bufs=2, space="PSUM"))
        psA = ctx.enter_context(tc.tile_pool(name="psA", bufs=1, space="PSUM"))
        psq = ctx.enter_context(tc.tile_pool(name="psq", bufs=1, space="PSUM"))
        psmom = ctx.enter_context(tc.tile_pool(name="psmom", bufs=1, space="PSUM"))

        # single activation-table load (sin + tanh live in silu_and_others)
        ld = mybir.InstLoadActFuncSet(
            act_func_set_id=SILU_SET_ID,
            name=nc.get_next_instruction_name(),
            engine=mybir.EngineType.Activation,
            ins=[],
            outs=[],
        )
        nc.scalar.add_instruction(ld)

        bigB_t = const.tile([128, BIGB], f16, tag="bigB", name="bigB_sb")
        nc.sync.dma_start(bigB_t[:], bigB[:, :])
        bigA_t = const.tile([128, BIGA], f16, tag="bigA", name="bigA_sb")
        nc.sync.dma_start(bigA_t[:], bigA[:, :])
        sT_t = const.tile([O, N], f16, tag="sT", name="sT_sb")
        nc.sync.dma_start(sT_t[:], sT[:, :])
        rows_t = const.tile([1, ROWS], f16, tag="rows", name="rows_sb")
        nc.sync.dma_start(rows_t[:], rows[:, :])
        bcols_t = const.tile([H, 4], f32, tag="bcols", name="bcols_sb")
        nc.sync.dma_start(bcols_t[:], bcols[:, :])
        bigC_t = const.tile([128, BIGC], f16, tag="bigC", name="bigC_sb")
        nc.sync.dma_start(bigC_t[:], bigC[:, :])

        zTi_s = bigA_t[:, A_ZTI : A_ZTI + 128]
        QK_s = bigA_t[:, A_QK : A_QK + 32]
        W1iT_s = bigB_t[:, B_W1I : B_W1I + 128]
        id_s = bigA_t[:, A_ID : A_ID + 128]
        zT_s = bigB_t[:, B_ZT : B_ZT + N]
        W1jT_s = bigB_t[:, B_W1J : B_W1J + 128]
        W2T3_s = bigC_t[:, C_W2T3 : C_W2T3 + 128]
        W2J3_s = bigC_t[:, C_W2J3 : C_W2J3 + 128]
        W2I3_s = bigC_t[:, C_W2I3 : C_W2I3 + 128]
        W4T_s = bigC_t[:, C_W4T : C_W4T + 128]
        ones_s = rows_t[:, R_ONES : R_ONES + 128]
        bit_s = rows_t[:, R_BIT : R_BIT + MH]
        bqk_s = rows_t[:, R_BQK : R_BQK + 32]
        b23_s = bcols_t[:, 0:1]
        b4_s = bcols_t[:, 1:2]
        blkv_s = bcols_t[:, 2:3]  # blk*128, per core

        hpi = work.tile([128, 1], f32, tag="hpi", name="hpi")
        nc.vector.memset(hpi[:], HALF_PI)

        # qkT[o, i] = (Wq.T@Wk/2sqrtH).T @ z_i.T + bqk  -> scores/2 = sT.T @ qkT
        qk_ps = psq.tile([32, 256], f16, tag="qk", name="qk_ps")
        qk32 = qk_ps[:, 0:256].bitcast(f32)
        nc.tensor.matmul(qk32, QK_s, zTi_s, start=True, stop=False)
        nc.tensor.matmul(qk32, bqk_s, ones_s, start=False, stop=True)
        qkT_t = work.tile([32, 128], f16, tag="qkT", name="qkT_sb")
        nc.vector.tensor_copy(qkT_t[:], qk32)

        # on-chip derived weights: RWj = [m w W1jT]_m, rit = [m w W1iT]_m
        RWj_t = work.tile([128, MH], f16, tag="RWj", name="RWj_sb")
        for m in range(M):
            nc.vector.tensor_scalar_mul(
                RWj_t[:, m * H : (m + 1) * H], W1jT_s, float((m + 1) * W)
            )
        rit_t = work.tile([128, MH], f16, tag="rit", name="rit_sb")
        for m in range(M):
            nc.vector.tensor_scalar_mul(
                rit_t[:, m * H : (m + 1) * H], W1iT_s, float((m + 1) * W)
            )
        idam_t = work.tile([128, M * 128], f16, tag="idam", name="idam_sb")
        for m in range(M):
            nc.vector.tensor_scalar_mul(
                idam_t[:, m * 128 : (m + 1) * 128], id_s, float(AM[m])
            )

        # F feature tiles [1 | sin | z | cos]; ones by memset, z by PE transpose
        F_t = []
        for c in range(NCH):
            fc = fpool.tile([128, NF], f16, tag="F", name=f"F{c}")
            F_t.append(fc)
        onec_t = work.tile([128, 1], f16, tag="onec", name="onec")
        nc.vector.memset(onec_t[:], 1.0)

        # iota d[p, col] = col - p  (for the diag mask compare)
        d_t = work.tile([128, N], i32, tag="d", name="d_sb")
        nc.gpsimd.iota(d_t[:], [[1, N]], base=0, channel_multiplier=-1)

        # scT[jj, c*128+i] = scores(i, j=c*128+jj)/2
        scT_ps = ps512.tile([128, N], f32, tag="b512", name="scT_ps")
        for c in range(NCH):
            nc.tensor.matmul(
                scT_ps[:, c * 128 : (c + 1) * 128],
                sT_t[:, c * 128 : (c + 1) * 128],
                qkT_t[:],
                start=True,
                stop=True,
            )
        th_t = work.tile([128, N], f32, tag="th", name="th_sb")
        nc.scalar.activation(th_t[:, 0:128], scT_ps[:, 0:128], AF.Tanh)
        nc.scalar.activation(th_t[:, 128:N], scT_ps[:, 128:N], AF.Tanh)

        # xi-side trig args: [m w xi]_m  (xi = z_i@W1iT + b1)
        xit_ps = ps512.tile([128, MH], f32, tag="b512", name="xit_ps")
        nc.tensor.matmul(xit_ps[:], zTi_s, rit_t[:], start=True, stop=False)
        nc.tensor.matmul(xit_ps[:], ones_s, bit_s, start=False, stop=True)
        xit_t = work.tile([128, MH], f32, tag="xit", name="xit_sb")
        nc.vector.tensor_copy(xit_t[:], xit_ps[:])

        # z-column blocks of F via PE transpose of zT chunks
        zt_ps = []
        for c in range(NCH):
            zp = psB.tile([128, 256], f16, tag="psB", name=f"zt{c}")
            nc.tensor.transpose(zp[:, 0:128], zT_s[:, c * 128 : (c + 1) * 128], id_s)
            zt_ps.append(zp)

        # E = exp(2*scT) = (1+th)/(1-th), diag zeroed; chunk 0 first so the
        # first moment matmul can start early.
        r1_t = work.tile([128, N], f32, tag="r1", name="r1")
        r2_t = work.tile([128, N], f32, tag="r2", name="r2")
        E_t = work.tile([128, N], f16, tag="E", name="E")
        for ci, (lo, hi) in enumerate(((0, 128), (128, N))):
            nc.vector.tensor_scalar(
                r1_t[:, lo:hi], th_t[:, lo:hi], -1.0, 1.0, ALU.mult, ALU.add
            )
            nc.vector.reciprocal_approx_fast(r2_t[:, lo:hi], r1_t[:, lo:hi])
            nc.vector.scalar_tensor_tensor(
                E_t[:, lo:hi], th_t[:, lo:hi], 1.0, r2_t[:, lo:hi],
                ALU.add, ALU.mult,
            )
            # E *= (d != blk*128): zeroes the attention diagonal
            nc.vector.scalar_tensor_tensor(
                E_t[:, lo:hi], d_t[:, lo:hi], blkv_s, E_t[:, lo:hi],
                ALU.not_equal, ALU.mult,
            )
            for c in ((0, 1) if ci == 0 else (2, 3)):
                nc.vector.tensor_copy(F_t[c][:, F_Z : F_Z + H], zt_ps[c][:, 0:128])

        # j-side features + moments, pipelined per chunk
        mom_ps = psmom.tile([128, 1024], f32, tag="mom", name="mom_ps")
        rs_t = work.tile([128, 1], f32, tag="rs", name="rs")
        slices = [(0, 512), (512, NF)]
        for c in range(NCH):
            xj_ps = ps512.tile([128, MH], f32, tag="b512", name=f"xj{c}")
            nc.tensor.matmul(
                xj_ps[:], zT_s[:, c * 128 : (c + 1) * 128], RWj_t[:],
                start=True, stop=True,
            )
            fc = F_t[c]
            nc.scalar.activation(fc[:, F_SIN : F_SIN + MH], xj_ps[:], AF.Sin)
            nc.scalar.activation(
                fc[:, F_COS : F_COS + MH], xj_ps[:], AF.Sin, bias=hpi[:, 0:1]
            )
            if c == 0:
                for cs in range(NCH):
                    nc.tensor.matmul(
                        mom_ps[:, 896:897],
                        E_t[:, cs * 128 : (cs + 1) * 128], onec_t[:],
                        start=(cs == 0), stop=(cs == NCH - 1),
                    )
                nc.vector.reciprocal(rs_t[:], mom_ps[:, 896:897])
            for s0, s1 in slices:
                nc.tensor.matmul(
                    mom_ps[:, s0:s1],
                    E_t[:, c * 128 : (c + 1) * 128],
                    fc[:, s0:s1],
                    start=(c == 0),
                    stop=(c == NCH - 1),
                )

        # xi-side trig (late in ACT queue: only needed by the combine)
        XiS = work.tile([128, MH], f16, tag="XiS", name="XiS")
        nc.scalar.activation(XiS[:], xit_t[:], AF.Sin)
        XiC = work.tile([128, MH], f16, tag="XiC", name="XiC")
        nc.scalar.activation(XiC[:], xit_t[:], AF.Sin, bias=hpi[:, 0:1])

        # combine: P = (XiS/ssum)*MCos + (XiC/ssum)*MSin; Tfin^T via
        # am-scaled-identity transpose-matmuls accumulating in PSUM.
        Mzn_t = work.tile([128, H], f16, tag="Mzn", name="Mzn")
        nc.scalar.activation(
            Mzn_t[:], mom_ps[:, F_Z : F_Z + H], AF.Identity, scale=rs_t[:, 0:1]
        )
        P2 = work.tile([128, MH], f16, tag="P2", name="P2")
        nc.vector.scalar_tensor_tensor(
            P2[:], XiC[:], rs_t[:, 0:1], mom_ps[:, F_SIN : F_SIN + MH],
            ALU.mult, ALU.mult,
        )
        P1 = work.tile([128, MH], f16, tag="P1", name="P1")
        nc.vector.scalar_tensor_tensor(
            P1[:], XiS[:], rs_t[:, 0:1], mom_ps[:, F_COS : F_COS + MH],
            ALU.mult, ALU.mult,
        )
        P = work.tile([128, MH], f16, tag="P", name="P")
        nc.vector.tensor_tensor(P[:], P1[:], P2[:], ALU.add)

        # epilogue, all in [h, i] layout; u = (W2@..@W3).T contributions
        u_ps = psA.tile([H, IPC], f32, tag="u", name="u_ps")
        nc.tensor.matmul(u_ps[:], W2I3_s, zTi_s, start=True, stop=False)
        mT_ps = psB.tile([128, 256], f16, tag="psB", name="mT_ps")
        nc.tensor.transpose(mT_ps[:, 0:128], Mzn_t[:], id_s)
        MzT = work.tile([128, IPC], f16, tag="MzT", name="MzT")
        nc.scalar.activation(MzT[:], mT_ps[:, 0:128], AF.Copy)
        PT_ps = psq.tile([128, 256], f16, tag="qk", name="PT_ps")
        PT32 = PT_ps[:, 0:256].bitcast(f32)
        for m in range(M):
            nc.tensor.matmul(
                PT32, P[:, m * H : (m + 1) * H],
                idam_t[:, m * 128 : (m + 1) * 128],
                start=(m == 0), stop=(m == M - 1),
            )
        acc_t = work.tile([128, IPC], f16, tag="acc", name="acc")
        nc.vector.tensor_copy(acc_t[:], PT32)
        nc.tensor.matmul(u_ps[:], W2J3_s, MzT[:], start=False, stop=False)
        nc.tensor.matmul(u_ps[:], W2T3_s, acc_t[:], start=False, stop=True)

        t3_t = work.tile([H, IPC], f16, tag="t3", name="t3_sb")
        nc.scalar.activation(t3_t[:], u_ps[:], AF.Tanh, bias=b23_s)
        dz_ps = psA.tile([H, IPC], f32, tag="u", name="dz_ps")
        nc.tensor.matmul(dz_ps[:], W4T_s, t3_t[:], start=True, stop=True)
        dzT = work.tile([H, IPC], f16, tag="dzT", name="dzT_sb")
        nc.scalar.activation(dzT[:], dz_ps[:], AF.Identity, bias=b4_s)
        for r in range(16):
            nc.sync.dma_start(out[8 * r : 8 * (r + 1), :], dzT[8 * r : 8 * (r + 1), :])

    nc.finalize()
    return nc


def _get_nc():
    if "nc" not in _CACHE:
        _CACHE["nc"] = _build()
    return _CACHE["nc"]


def kernel(**inputs):
    global LAST_RESULTS
    from concourse.bass_utils import run_bass_kernel_spmd

    f = np.float32
    z = np.asarray(inputs["z"], f)
    s_t = np.asarray(inputs["s_t"], f)
    W1 = np.asarray(inputs["W1"], f)
    b1 = np.asarray(inputs["b1"], f)
    W2 = np.asarray(inputs["W2"], f)
    b2 = np.asarray(inputs["b2"], f)
    Wq = np.asarray(inputs["Wq"], f)
    bq = np.asarray(inputs["bq"], f)
    Wk = np.asarray(inputs["Wk"], f)
    W3 = np.asarray(inputs["W3"], f)
    b3 = np.asarray(inputs["b3"], f)
    W4 = np.asarray(inputs["W4"], f)
    b4 = np.asarray(inputs["b4"], f)

    h16 = np.float16
    tr = lambda m: np.ascontiguousarray(m.T, f)

    rt = f(1.0 / (2.0 * np.sqrt(H)))
    W1iT = tr(W1[:, :H])
    W1jT = tr(W1[:, H:])
    W2T = tr(W2)
    W3T = tr(W3)
    QKmat = (Wq.T @ Wk) * rt
    bqk = (bq @ Wk) * rt
    brow_it = np.concatenate([(m + 1) * W * b1 for m in range(M)])
    W2T3 = W2T @ W3T
    W2J3 = (LIN_C * (W1jT @ W2T)) @ W3T
    W2I3 = (LIN_C * (W1iT @ W2T)) @ W3T
    b23 = (b2 + LIN_C * (b1 @ W2T)) @ W3T + b3

    rows = np.zeros((1, ROWS), h16)
    rows[0, R_ONES : R_ONES + 128] = 1.0
    rows[0, R_BIT : R_BIT + MH] = brow_it.astype(h16)
    rows[0, R_BQK : R_BQK + 32] = bqk.astype(h16)

    bigA_shared = np.zeros((128, BIGA), h16)
    bigA_shared[:, A_QK : A_QK + 32] = QKmat.astype(h16)
    bigA_shared[:, A_ID : A_ID + 128] = np.eye(128, dtype=h16)
    bigC = np.zeros((128, BIGC), h16)
    bigC[:, C_W2T3 : C_W2T3 + 128] = W2T3.astype(h16)
    bigC[:, C_W2J3 : C_W2J3 + 128] = W2J3.astype(h16)
    bigC[:, C_W2I3 : C_W2I3 + 128] = W2I3.astype(h16)
    bigC[:, C_W4T : C_W4T + 128] = tr(W4).astype(h16)

    in_maps = []
    for c in range(NC):
        b, blk = divmod(c, CPB)
        i0 = blk * IPC
        bigA = bigA_shared.copy()
        bigA[:, A_ZTI : A_ZTI + 128] = z[b, i0 : i0 + IPC].T.astype(h16)
        bigB = np.zeros((128, BIGB), h16)
        bigB[:, B_ZT : B_ZT + N] = z[b].T.astype(h16)
        bigB[:, B_W1J : B_W1J + 128] = W1jT.astype(h16)
        bigB[:, B_W1I : B_W1I + 128] = W1iT.astype(h16)
        bcols = np.zeros((H, 4), f)
        bcols[:, 0] = b23
        bcols[:, 1] = b4
        bcols[:, 2] = blk * 128
        in_maps.append(
            dict(
                bigA=bigA,
                sT=s_t[b].T.astype(h16),
                rows=rows,
                bigB=bigB,
                bigC=bigC,
                bcols=bcols,
            )
        )

    nc = _get_nc()
    res = run_bass_kernel_spmd(nc, in_maps, core_ids=list(range(NC)))
    LAST_RESULTS = res

    dz = np.empty((B, N, H), dtype=f)
    for c in range(NC):
        b, blk = divmod(c, CPB)
        i0 = blk * IPC
        dz[b, i0 : i0 + IPC, :] = res.results[c]["out"].T.astype(f)
    return dz


# revision 22
# speedup vs baseline: 1.1813x; 1.0192x over previous
"""Trainium2 Bass kernel for the ODEFunc GNN message-passing module.

Math (B=2, N=512, H=128, O=32):
    q = z @ Wq.T + bq ;  k = s_t @ Wk.T + bk
    scores = (q @ k.T)/sqrt(H), diagonal masked to -inf
    attn = softmax_j(scores)
    U    = sum_j attn[i,j] * tanh(xi_i + yj_j)      (xi = z@W1i.T + b1, yj = z@W1j.T)
    agg  = U @ W2.T + b2     (softmax rows sum to 1 -> W2 moves after aggregation)
    dz   = tanh(agg @ W3.T + b3) @ W4.T + b4

Key trick: expand tanh in a factorized basis
    tanh(x) ~ LIN_C*x + sum_m AM[m]*sin(m*W*x)        on |x| <= 4.35
so with sin(m w (xi+yj)) = sin(m w xi)cos(m w yj) + cos(m w xi)sin(m w yj),
the attention aggregation becomes moment matmuls E^T @ [1 | z | sin | cos]
with E[j,i] = exp(scores) (unnormalized, diag-zeroed).  The xi-linear and
z-moment-linear terms fold into extra epilogue matmuls; W3 is folded into
the W2-stage matrices (W2?3 = W2? @ W3T) so the epilogue is two matmul
stages; 1/ssum folds into the combine via the ones-column moment.

exp(s) = (1+tanh(s/2))/(1-tanh(s/2)) so sin+tanh suffice -> a single
manually-placed LoadActFuncSet(silu_and_others) covers every activation.
q/k projections fold into one [H,O] matrix (bk cancels in softmax).
On-chip derivations minimize input DMA: diag mask via iota+compare, the
m-scaled weight blocks via DVE scalar muls, F's z-columns via PE
transposes of zT.  All matmul operands fp16; fp32 PSUM accumulation.

Sharding: 1024 (b,i) pairs over 8 cores (batch-major, 128 i's per core).
"""

import numpy as np

B, N, H, O = 2, 512, 128, 32
NC = 8
CPB = NC // B  # cores per batch = 4
IPC = N // CPB  # i's per core = 128
NCH = N // 128  # j chunks = 4

# tanh(x) ~ LIN_C*x + sum_m AM[m] sin((m+1) W x), minimax fit on [-4.35, 4.35]
W = 0.9130
LIN_C = 0.289778
AM = [0.463016, 0.103367, 0.026572]
M = 3
MH = M * H  # 384
NF = 1 + H + 2 * MH  # 897 feature cols: [1 | sin | z | cos]
HALF_PI = 1.5707963267948966
SILU_SET_ID = 18  # silu_and_others: contains both sin and tanh

# bigA packed columns (fp16, [128, .]) -- scores path
A_ZTI = 0             # zTi   [H, 128]
A_QK = 128            # QKmat [H, 32]
A_ID = 160            # identity [128, 128]
BIGA = 288
# bigB packed columns -- feature path (DMA'd first: heads the critical chain)
B_ZT = 0              # zT    [H, N]
B_W1J = N             # W1jT  [H, H]
B_W1I = N + 128       # W1iT  [H, H]
BIGB = N + 256
# F feature column layout: [1 | sin | z | cos]
F_SIN = 1
F_Z = 1 + MH
F_COS = 1 + MH + H
# bigC packed columns -- epilogue weights
C_W2T3 = 0            # W2T@W3T [H, H]
C_W2J3 = 128          # W2J@W3T
C_W2I3 = 256          # W2I@W3T
C_W4T = 384           # W4T
BIGC = 512
# rows packed (fp16, [1, .])
R_ONES = 0
R_BIT = 128           # brow_it [1, MH]
R_BQK = 128 + MH      # bqk [1, 32]
ROWS = 160 + MH

_CACHE = {}

# Stash of the last BassKernelResults (exec_time_ns etc.) for test harnesses.
LAST_RESULTS = None


def _build():
    from contextlib import ExitStack

    import concourse.tile as tile
    from concourse import bacc, mybir

    f32 = mybir.dt.float32
    f16 = mybir.dt.float16
    i32 = mybir.dt.int32
    AF = mybir.ActivationFunctionType
    ALU = mybir.AluOpType

    nc = bacc.Bacc(trn_type="TRN2")

    bigA = nc.dram_tensor("bigA", [128, BIGA], f16, kind="ExternalInput")
    sT = nc.dram_tensor("sT", [O, N], f16, kind="ExternalInput")
    rows = nc.dram_tensor("rows", [1, ROWS], f16, kind="ExternalInput")
    bigB = nc.dram_tensor("bigB", [128, BIGB], f16, kind="ExternalInput")
    bigC = nc.dram_tensor("bigC", [128, BIGC], f16, kind="ExternalInput")
    bcols = nc.dram_tensor("bcols", [H, 4], f32, kind="ExternalInput")
    out = nc.dram_tensor("out", [H, IPC], f16, kind="ExternalOutput")

    with tile.TileContext(nc) as tc, ExitStack() as ctx:
        const = ctx.enter_context(tc.tile_pool(name="const", bufs=1))
        work = ctx.enter_context(tc.tile_pool(name="work", bufs=1))
        fpool = ctx.enter_context(tc.tile_pool(name="fpool", bufs=4))
        ps512 = ctx.enter_context(tc.tile_pool(name="ps512", bufs=2, space="PSUM"))
        psB = ctx.enter_context(tc.tile_pool(name="psB", bufs=2, space="PSUM"))
        psA = ctx.enter_context(tc.tile_pool(name="psA", bufs=1, space="PSUM"))
        psq = ctx.enter_context(tc.tile_pool(name="psq", bufs=1, space="PSUM"))
        psmom = ctx.enter_context(tc.tile_pool(name="psmom", bufs=1, space="PSUM"))

        # single activation-table load (sin + tanh live in silu_and_others)
        ld = mybir.InstLoadActFuncSet(
            act_func_set_id=SILU_SET_ID,
            name=nc.get_next_instruction_name(),
            engine=mybir.EngineType.Activation,
            ins=[],
            outs=[],
        )
        nc.scalar.add_instruction(ld)

        bigB_t = const.tile([128, BIGB], f16, tag="bigB", name="bigB_sb")
        nc.sync.dma_start(bigB_t[:], bigB[:, :])
        bigA_t = const.tile([128, BIGA], f16, tag="bigA", name="bigA_sb")
        nc.sync.dma_start(bigA_t[:], bigA[:, :])
        sT_t = const.tile([O, N], f16, tag="sT", name="sT_sb")
        nc.sync.dma_start(sT_t[:], sT[:, :])
        rows_t = const.tile([1, ROWS], f16, tag="rows", name="rows_sb")
        nc.sync.dma_start(rows_t[:], rows[:, :])
        bcols_t = const.tile([H, 4], f32, tag="bcols", name="bcols_sb")
        nc.sync.dma_start(bcols_t[:], bcols[:, :])
        bigC_t = const.tile([128, BIGC], f16, tag="bigC", name="bigC_sb")
        nc.sync.dma_start(bigC_t[:], bigC[:, :])

        zTi_s = bigA_t[:, A_ZTI : A_ZTI + 128]
        QK_s = bigA_t[:, A_QK : A_QK + 32]
        W1iT_s = bigB_t[:, B_W1I : B_W1I + 128]
        id_s = bigA_t[:, A_ID : A_ID + 128]
        zT_s = bigB_t[:, B_ZT : B_ZT + N]
        W1jT_s = bigB_t[:, B_W1J : B_W1J + 128]
        W2T3_s = bigC_t[:, C_W2T3 : C_W2T3 + 128]
        W2J3_s = bigC_t[:, C_W2J3 : C_W2J3 + 128]
        W2I3_s = bigC_t[:, C_W2I3 : C_W2I3 + 128]
        W4T_s = bigC_t[:, C_W4T : C_W4T + 128]
        ones_s = rows_t[:, R_ONES : R_ONES + 128]
        bit_s = rows_t[:, R_BIT : R_BIT + MH]
        bqk_s = rows_t[:, R_BQK : R_BQK + 32]
        b23_s = bcols_t[:, 0:1]
        b4_s = bcols_t[:, 1:2]
        blkv_s = bcols_t[:, 2:3]  # blk*128, per core

        hpi = work.tile([128, 1], f32, tag="hpi", name="hpi")
        nc.vector.memset(hpi[:], HALF_PI)

        # qkT[o, i] = (Wq.T@Wk/2sqrtH).T @ z_i.T + bqk  -> scores/2 = sT.T @ qkT
        qk_ps = psq.tile([32, 256], f16, tag="qk", name="qk_ps")
        qk32 = qk_ps[:, 0:256].bitcast(f32)
        nc.tensor.matmul(qk32, QK_s, zTi_s, start=True, stop=False)
        nc.tensor.matmul(qk32, bqk_s, ones_s, start=False, stop=True)
        qkT_t = work.tile([32, 128], f16, tag="qkT", name="qkT_sb")
        nc.vector.tensor_copy(qkT_t[:], qk32)

        # on-chip derived weights: RWj = [m w W1jT]_m, rit = [m w W1iT]_m
        RWj_t = work.tile([128, MH], f16, tag="RWj", name="RWj_sb")
        for m in range(M):
            nc.vector.tensor_scalar_mul(
                RWj_t[:, m * H : (m + 1) * H], W1jT_s, float((m + 1) * W)
            )
        rit_t = work.tile([128, MH], f16, tag="rit", name="rit_sb")
        for m in range(M):
            nc.vector.tensor_scalar_mul(
                rit_t[:, m * H : (m + 1) * H], W1iT_s, float((m + 1) * W)
            )
        idam_t = work.tile([128, M * 128], f16, tag="idam", name="idam_sb")
        for m in range(M):
            nc.vector.tensor_scalar_mul(
                idam_t[:, m * 128 : (m + 1) * 128], id_s, float(AM[m])
            )

        # F feature tiles [1 | sin | z | cos]; ones by memset, z by PE transpose
        F_t = []
        for c in range(NCH):
            fc = fpool.tile([128, NF], f16, tag="F", name=f"F{c}")
            nc.vector.memset(fc[:, 0:1], 1.0)
            F_t.append(fc)

        # iota d[p, col] = col - p  (for the diag mask compare)
        d_t = work.tile([128, N], i32, tag="d", name="d_sb")
        nc.gpsimd.iota(d_t[:], [[1, N]], base=0, channel_multiplier=-1)

        # scT[jj, c*128+i] = scores(i, j=c*128+jj)/2
        scT_ps = ps512.tile([128, N], f32, tag="b512", name="scT_ps")
        for c in range(NCH):
            nc.tensor.matmul(
                scT_ps[:, c * 128 : (c + 1) * 128],
                sT_t[:, c * 128 : (c + 1) * 128],
                qkT_t[:],
                start=True,
                stop=True,
            )
        th_t = work.tile([128, N], f32, tag="th", name="th_sb")
        nc.scalar.activation(th_t[:, 0:128], scT_ps[:, 0:128], AF.Tanh)
        nc.scalar.activation(th_t[:, 128:N], scT_ps[:, 128:N], AF.Tanh)

        # xi-side trig args: [m w xi]_m  (xi = z_i@W1iT + b1)
        xit_ps = ps512.tile([128, MH], f32, tag="b512", name="xit_ps")
        nc.tensor.matmul(xit_ps[:], zTi_s, rit_t[:], start=True, stop=False)
        nc.tensor.matmul(xit_ps[:], ones_s, bit_s, start=False, stop=True)
        xit_t = work.tile([128, MH], f32, tag="xit", name="xit_sb")
        nc.vector.tensor_copy(xit_t[:], xit_ps[:])

        # z-column blocks of F via PE transpose of zT chunks
        zt_ps = []
        for c in range(NCH):
            zp = psB.tile([128, 256], f16, tag="psB", name=f"zt{c}")
            nc.tensor.transpose(zp[:, 0:128], zT_s[:, c * 128 : (c + 1) * 128], id_s)
            zt_ps.append(zp)

        # E = exp(2*scT) = (1+th)/(1-th), diag zeroed; chunk 0 first so the
        # first moment matmul can start early.
        r1_t = work.tile([128, N], f32, tag="r1", name="r1")
        r2_t = work.tile([128, N], f32, tag="r2", name="r2")
        E_t = work.tile([128, N], f16, tag="E", name="E")
        for ci, (lo, hi) in enumerate(((0, 128), (128, N))):
            nc.vector.tensor_scalar(
                r1_t[:, lo:hi], th_t[:, lo:hi], -1.0, 1.0, ALU.mult, ALU.add
            )
            nc.vector.reciprocal_approx_fast(r2_t[:, lo:hi], r1_t[:, lo:hi])
            nc.vector.scalar_tensor_tensor(
                E_t[:, lo:hi], th_t[:, lo:hi], 1.0, r2_t[:, lo:hi],
                ALU.add, ALU.mult,
            )
            # E *= (d != blk*128): zeroes the attention diagonal
            nc.vector.scalar_tensor_tensor(
                E_t[:, lo:hi], d_t[:, lo:hi], blkv_s, E_t[:, lo:hi],
                ALU.not_equal, ALU.mult,
            )
            for c in ((0, 1) if ci == 0 else (2, 3)):
                nc.vector.tensor_copy(F_t[c][:, F_Z : F_Z + H], zt_ps[c][:, 0:128])

        # j-side features + moments, pipelined per chunk
        mom_ps = psmom.tile([128, 1024], f32, tag="mom", name="mom_ps")
        slices = [(0, 512), (512, NF)]
        for c in range(NCH):
            xj_ps = ps512.tile([128, MH], f32, tag="b512", name=f"xj{c}")
            nc.tensor.matmul(
                xj_ps[:], zT_s[:, c * 128 : (c + 1) * 128], RWj_t[:],
                start=True, stop=True,
            )
            fc = F_t[c]
            nc.scalar.activation(fc[:, F_SIN : F_SIN + MH], xj_ps[:], AF.Sin)
            nc.scalar.activation(
                fc[:, F_COS : F_COS + MH], xj_ps[:], AF.Sin, bias=hpi[:, 0:1]
            )
            for s0, s1 in slices:
                nc.tensor.matmul(
                    mom_ps[:, s0:s1],
                    E_t[:, c * 128 : (c + 1) * 128],
                    fc[:, s0:s1],
                    start=(c == 0),
                    stop=(c == NCH - 1),
                )

        # xi-side trig (late in ACT queue: only needed by the combine)
        XiS = work.tile([128, MH], f16, tag="XiS", name="XiS")
        nc.scalar.activation(XiS[:], xit_t[:], AF.Sin)
        XiC = work.tile([128, MH], f16, tag="XiC", name="XiC")
        nc.scalar.activation(XiC[:], xit_t[:], AF.Sin, bias=hpi[:, 0:1])

        # combine: P = (XiS/ssum)*MCos + (XiC/ssum)*MSin; Tfin^T via
        # am-scaled-identity transpose-matmuls accumulating in PSUM.
        rs_t = work.tile([128, 1], f32, tag="rs", name="rs")
        nc.vector.reciprocal(rs_t[:], mom_ps[:, 0:1])
        Mzn_t = work.tile([128, H], f16, tag="Mzn", name="Mzn")
        nc.scalar.activation(
            Mzn_t[:], mom_ps[:, F_Z : F_Z + H], AF.Identity, scale=rs_t[:, 0:1]
        )
        P2 = work.tile([128, MH], f16, tag="P2", name="P2")
        nc.vector.scalar_tensor_tensor(
            P2[:], XiC[:], rs_t[:, 0:1], mom_ps[:, F_SIN : F_SIN + MH],
            ALU.mult, ALU.mult,
        )
        P1 = work.tile([128, MH], f16, tag="P1", name="P1")
        nc.vector.scalar_tensor_tensor(
            P1[:], XiS[:], rs_t[:, 0:1], mom_ps[:, F_COS : F_COS + MH],
            ALU.mult, ALU.mult,
        )
        P = work.tile([128, MH], f16, tag="P", name="P")
        nc.vector.tensor_tensor(P[:], P1[:], P2[:], ALU.add)

        # epilogue, all in [h, i] layout; u = (W2@..@W3).T contributions
        u_ps = psA.tile([H, IPC], f32, tag="u", name="u_ps")
        nc.tensor.matmul(u_ps[:], W2I3_s, zTi_s, start=True, stop=False)
        mT_ps = psB.tile([128, 256], f16, tag="psB", name="mT_ps")
        nc.tensor.transpose(mT_ps[:, 0:128], Mzn_t[:], id_s)
        MzT = work.tile([128, IPC], f16, tag="MzT", name="MzT")
        nc.scalar.activation(MzT[:], mT_ps[:, 0:128], AF.Copy)
        PT_ps = psq.tile([128, 256], f16, tag="qk", name="PT_ps")
        PT32 = PT_ps[:, 0:256].bitcast(f32)
        for m in range(M):
            nc.tensor.matmul(
                PT32, P[:, m * H : (m + 1) * H],
                idam_t[:, m * 128 : (m + 1) * 128],
                start=(m == 0), stop=(m == M - 1),
            )
        acc_t = work.tile([128, IPC], f16, tag="acc", name="acc")
        nc.vector.tensor_copy(acc_t[:], PT32)
        nc.tensor.matmul(u_ps[:], W2J3_s, MzT[:], start=False, stop=False)
        nc.tensor.matmul(u_ps[:], W2T3_s, acc_t[:], start=False, stop=True)

        t3_t = work.tile([H, IPC], f16, tag="t3", name="t3_sb")
        nc.scalar.activation(t3_t[:], u_ps[:], AF.Tanh, bias=b23_s)
        dz_ps = psA.tile([H, IPC], f32, tag="u", name="dz_ps")
        nc.tensor.matmul(dz_ps[:], W4T_s, t3_t[:], start=True, stop=True)
        dzT = work.tile([H, IPC], f16, tag="dzT", name="dzT_sb")
        nc.scalar.activation(dzT[:], dz_ps[:], AF.Identity, bias=b4_s)
        for r in range(16):
            nc.sync.dma_start(out[8 * r : 8 * (r + 1), :], dzT[8 * r : 8 * (r + 1), :])

    nc.finalize()
    return nc


def _get_nc():
    if "nc" not in _CACHE:
        _CACHE["nc"] = _build()
    return _CACHE["nc"]


def kernel(**inputs):
    global LAST_RESULTS
    from concourse.bass_utils import run_bass_kernel_spmd

    f = np.float32
    z = np.asarray(inputs["z"], f)
    s_t = np.asarray(inputs["s_t"], f)
    W1 = np.asarray(inputs["W1"], f)
    b1 = np.asarray(inputs["b1"], f)
    W2 = np.asarray(inputs["W2"], f)
    b2 = np.asarray(inputs["b2"], f)
    Wq = np.asarray(inputs["Wq"], f)
    bq = np.asarray(inputs["bq"], f)
    Wk = np.asarray(inputs["Wk"], f)
    W3 = np.asarray(inputs["W3"], f)
    b3 = np.asarray(inputs["b3"], f)
    W4 = np.asarray(inputs["W4"], f)
    b4 = np.asarray(inputs["b4"], f)

    h16 = np.float16
    tr = lambda m: np.ascontiguousarray(m.T, f)

    rt = f(1.0 / (2.0 * np.sqrt(H)))
    W1iT = tr(W1[:, :H])
    W1jT = tr(W1[:, H:])
    W2T = tr(W2)
    W3T = tr(W3)
    QKmat = (Wq.T @ Wk) * rt
    bqk = (bq @ Wk) * rt
    brow_it = np.concatenate([(m + 1) * W * b1 for m in range(M)])
    W2T3 = W2T @ W3T
    W2J3 = (LIN_C * (W1jT @ W2T)) @ W3T
    W2I3 = (LIN_C * (W1iT @ W2T)) @ W3T
    b23 = (b2 + LIN_C * (b1 @ W2T)) @ W3T + b3

    rows = np.zeros((1, ROWS), h16)
    rows[0, R_ONES : R_ONES + 128] = 1.0
    rows[0, R_BIT : R_BIT + MH] = brow_it.astype(h16)
    rows[0, R_BQK : R_BQK + 32] = bqk.astype(h16)

    bigA_shared = np.zeros((128, BIGA), h16)
    bigA_shared[:, A_QK : A_QK + 32] = QKmat.astype(h16)
    bigA_shared[:, A_ID : A_ID + 128] = np.eye(128, dtype=h16)
    bigC = np.zeros((128, BIGC), h16)
    bigC[:, C_W2T3 : C_W2T3 + 128] = W2T3.astype(h16)
    bigC[:, C_W2J3 : C_W2J3 + 128] = W2J3.astype(h16)
    bigC[:, C_W2I3 : C_W2I3 + 128] = W2I3.astype(h16)
    bigC[:, C_W4T : C_W4T + 128] = tr(W4).astype(h16)

    in_maps = []
    for c in range(NC):
        b, blk = divmod(c, CPB)
        i0 = blk * IPC
        bigA = bigA_shared.copy()
        bigA[:, A_ZTI : A_ZTI + 128] = z[b, i0 : i0 + IPC].T.astype(h16)
        bigB = np.zeros((128, BIGB), h16)
        bigB[:, B_ZT : B_ZT + N] = z[b].T.astype(h16)
        bigB[:, B_W1J : B_W1J + 128] = W1jT.astype(h16)
        bigB[:, B_W1I : B_W1I + 128] = W1iT.astype(h16)
        bcols = np.zeros((H, 4), f)
        bcols[:, 0] = b23
        bcols[:, 1] = b4
        bcols[:, 2] = blk * 128
        in_maps.append(
            dict(
                bigA=bigA,
                sT=s_t[b].T.astype(h16),
                rows=rows,
                bigB=bigB,
                bigC=bigC,
                bcols=bcols,
            )
        )

    nc = _get_nc()
    res = run_bass_kernel_spmd(nc, in_maps, core_ids=list(range(NC)))
    LAST_RESULTS = res

    dz = np.empty((B, N, H), dtype=f)
    for c in range(NC):
        b, blk = divmod(c, CPB)
        i0 = blk * IPC
        dz[b, i0 : i0 + IPC, :] = res.results[c]["out"].T.astype(f)
    return dz


# revision 23
# speedup vs baseline: 1.5350x; 1.2995x over previous
"""Trainium2 Bass kernel for the ODEFunc GNN message-passing module.

Math (B=2, N=512, H=128, O=32):
    q = z @ Wq.T + bq ;  k = s_t @ Wk.T + bk
    scores = (q @ k.T)/sqrt(H), diagonal masked to -inf
    attn = softmax_j(scores)
    U    = sum_j attn[i,j] * tanh(xi_i + yj_j)      (xi = z@W1i.T + b1, yj = z@W1j.T)
    agg  = U @ W2.T + b2     (softmax rows sum to 1 -> W2 moves after aggregation)
    dz   = tanh(agg @ W3.T + b3) @ W4.T + b4

Key trick: expand tanh in a factorized basis
    tanh(x) ~ LIN_C*x + sum_m AM[m]*sin(m*W*x)        on |x| <= 4.35
so with sin(m w (xi+yj)) = sin(m w xi)cos(m w yj) + cos(m w xi)sin(m w yj),
the attention aggregation becomes moment matmuls E^T @ [1 | z | sin | cos]
with E[j,i] = exp(scores) (unnormalized, diag-zeroed).  The xi-linear and
z-moment-linear terms fold into extra epilogue matmuls; W3 is folded into
the W2-stage matrices (W2?3 = W2? @ W3T) so the epilogue is two matmul
stages; 1/ssum folds into the combine via the ones-column moment.

exp(s) = (1+tanh(s/2))/(1-tanh(s/2)) so sin+tanh suffice -> a single
manually-placed LoadActFuncSet(silu_and_others) covers every activation.
q/k projections fold into one [H,O] matrix (bk cancels in softmax).
On-chip derivations minimize input DMA: diag mask via iota+compare, the
m-scaled weight blocks via DVE scalar muls, F's z-columns via PE
transposes of zT.  All matmul operands fp16; fp32 PSUM accumulation.

Sharding: 1024 (b,i) pairs over 8 cores (batch-major, 128 i's per core).
"""

import numpy as np

B, N, H, O = 2, 512, 128, 32
NC = 8
CPB = NC // B  # cores per batch = 4
IPC = N // CPB  # i's per core = 128
NCH = N // 128  # j chunks = 4

# tanh(x) ~ LIN_C*x + sum_m AM[m] sin((m+1) W x), minimax fit on [-4.35, 4.35]
W = 0.9130
LIN_C = 0.289778
AM = [0.463016, 0.103367, 0.026572]
M = 3
MH = M * H  # 384
NF = 1 + H + 2 * MH  # 897 feature cols: [1 | sin | z | cos]
HALF_PI = 1.5707963267948966
SILU_SET_ID = 18  # silu_and_others: contains both sin and tanh

# bigA packed columns (fp16, [128, .]) -- everything the front needs
A_ZTI = 0             # zTi   [H, 128]
A_QK = 128            # QKmat [H, 32]
A_ID = 160            # identity [128, 128]
A_W1I = 288           # W1iT  [H, H]
A_W1J = 416           # W1jT  [H, H]
BIGA = 544
# bigB -- just zT
B_ZT = 0              # zT    [H, N]
BIGB = N
# F feature column layout: [1 | sin | z | cos]
F_SIN = 1
F_Z = 1 + MH
F_COS = 1 + MH + H
# bigC packed columns -- epilogue weights
C_W2T3 = 0            # W2T@W3T [H, H]
C_W2J3 = 128          # W2J@W3T
C_W2I3 = 256          # W2I@W3T
C_W4T = 384           # W4T
BIGC = 512
# rows packed (fp16, [1, .])
R_ONES = 0
R_BIT = 128           # brow_it [1, MH]
R_BQK = 128 + MH      # bqk [1, 32]
ROWS = 160 + MH

_CACHE = {}

# Stash of the last BassKernelResults (exec_time_ns etc.) for test harnesses.
LAST_RESULTS = None


def _build():
    from contextlib import ExitStack

    import concourse.tile as tile
    from concourse import bacc, mybir

    f32 = mybir.dt.float32
    f16 = mybir.dt.float16
    i32 = mybir.dt.int32
    AF = mybir.ActivationFunctionType
    ALU = mybir.AluOpType

    nc = bacc.Bacc(trn_type="TRN2")

    bigA = nc.dram_tensor("bigA", [128, BIGA], f16, kind="ExternalInput")
    sT = nc.dram_tensor("sT", [O, N], f16, kind="ExternalInput")
    rows = nc.dram_tensor("rows", [1, ROWS], f16, kind="ExternalInput")
    bigB = nc.dram_tensor("bigB", [128, BIGB], f16, kind="ExternalInput")
    bigC = nc.dram_tensor("bigC", [128, BIGC], f16, kind="ExternalInput")
    bcols = nc.dram_tensor("bcols", [H, 4], f32, kind="ExternalInput")
    out = nc.dram_tensor("out", [H, IPC], f16, kind="ExternalOutput")

    with tile.TileContext(nc) as tc, ExitStack() as ctx:
        const = ctx.enter_context(tc.tile_pool(name="const", bufs=1))
        work = ctx.enter_context(tc.tile_pool(name="work", bufs=1))
        fpool = ctx.enter_context(tc.tile_pool(name="fpool", bufs=4))
        ps512 = ctx.enter_context(tc.tile_pool(name="ps512", bufs=2, space="PSUM"))
        psB = ctx.enter_context(tc.tile_pool(name="psB", bufs=2, space="PSUM"))
        psA = ctx.enter_context(tc.tile_pool(name="psA", bufs=1, space="PSUM"))
        psq = ctx.enter_context(tc.tile_pool(name="psq", bufs=1, space="PSUM"))
        psmom = ctx.enter_context(tc.tile_pool(name="psmom", bufs=1, space="PSUM"))

        # single activation-table load (sin + tanh live in silu_and_others)
        ld = mybir.InstLoadActFuncSet(
            act_func_set_id=SILU_SET_ID,
            name=nc.get_next_instruction_name(),
            engine=mybir.EngineType.Activation,
            ins=[],
            outs=[],
        )
        nc.scalar.add_instruction(ld)

        bigA_t = const.tile([128, BIGA], f16, tag="bigA", name="bigA_sb")
        nc.sync.dma_start(bigA_t[:], bigA[:, :])
        bigB_t = const.tile([128, BIGB], f16, tag="bigB", name="bigB_sb")
        nc.sync.dma_start(bigB_t[:], bigB[:, :])
        sT_t = const.tile([O, N], f16, tag="sT", name="sT_sb")
        nc.sync.dma_start(sT_t[:], sT[:, :])
        rows_t = const.tile([1, ROWS], f16, tag="rows", name="rows_sb")
        nc.sync.dma_start(rows_t[:], rows[:, :])
        bcols_t = const.tile([H, 4], f32, tag="bcols", name="bcols_sb")
        nc.sync.dma_start(bcols_t[:], bcols[:, :])
        bigC_t = const.tile([128, BIGC], f16, tag="bigC", name="bigC_sb")
        nc.sync.dma_start(bigC_t[:], bigC[:, :])

        zTi_s = bigA_t[:, A_ZTI : A_ZTI + 128]
        QK_s = bigA_t[:, A_QK : A_QK + 32]
        W1iT_s = bigA_t[:, A_W1I : A_W1I + 128]
        id_s = bigA_t[:, A_ID : A_ID + 128]
        zT_s = bigB_t[:, B_ZT : B_ZT + N]
        W1jT_s = bigA_t[:, A_W1J : A_W1J + 128]
        W2T3_s = bigC_t[:, C_W2T3 : C_W2T3 + 128]
        W2J3_s = bigC_t[:, C_W2J3 : C_W2J3 + 128]
        W2I3_s = bigC_t[:, C_W2I3 : C_W2I3 + 128]
        W4T_s = bigC_t[:, C_W4T : C_W4T + 128]
        ones_s = rows_t[:, R_ONES : R_ONES + 128]
        bit_s = rows_t[:, R_BIT : R_BIT + MH]
        bqk_s = rows_t[:, R_BQK : R_BQK + 32]
        b23_s = bcols_t[:, 0:1]
        b4_s = bcols_t[:, 1:2]
        blkv_s = bcols_t[:, 2:3]  # blk*128, per core

        hpi = work.tile([128, 1], f32, tag="hpi", name="hpi")
        nc.vector.memset(hpi[:], HALF_PI)

        # qkT[o, i] = (Wq.T@Wk/2sqrtH).T @ z_i.T + bqk  -> scores/2 = sT.T @ qkT
        qk_ps = psq.tile([32, 256], f16, tag="qk", name="qk_ps")
        qk32 = qk_ps[:, 0:256].bitcast(f32)
        nc.tensor.matmul(qk32, QK_s, zTi_s, start=True, stop=False)
        nc.tensor.matmul(qk32, bqk_s, ones_s, start=False, stop=True)
        qkT_t = work.tile([32, 128], f16, tag="qkT", name="qkT_sb")
        nc.vector.tensor_copy(qkT_t[:], qk32)

        # on-chip derived weights: RWj = [m w W1jT]_m, rit = [m w W1iT]_m
        RWj_t = work.tile([128, MH], f16, tag="RWj", name="RWj_sb")
        for m in range(M):
            nc.vector.tensor_scalar_mul(
                RWj_t[:, m * H : (m + 1) * H], W1jT_s, float((m + 1) * W)
            )
        rit_t = work.tile([128, MH], f16, tag="rit", name="rit_sb")
        for m in range(M):
            nc.vector.tensor_scalar_mul(
                rit_t[:, m * H : (m + 1) * H], W1iT_s, float((m + 1) * W)
            )
        idam_t = work.tile([128, M * 128], f16, tag="idam", name="idam_sb")
        for m in range(M):
            nc.vector.tensor_scalar_mul(
                idam_t[:, m * 128 : (m + 1) * 128], id_s, float(AM[m])
            )

        # F feature tiles [1 | sin | z | cos]; ones by memset, z by PE transpose
        F_t = []
        for c in range(NCH):
            fc = fpool.tile([128, NF], f16, tag="F", name=f"F{c}")
            nc.vector.memset(fc[:, 0:1], 1.0)
            F_t.append(fc)

        # iota d[p, col] = col - p  (for the diag mask compare)
        d_t = work.tile([128, N], i32, tag="d", name="d_sb")
        nc.gpsimd.iota(d_t[:], [[1, N]], base=0, channel_multiplier=-1)

        # scT[jj, c*128+i] = scores(i, j=c*128+jj)/2
        scT_ps = ps512.tile([128, N], f32, tag="b512", name="scT_ps")
        for c in range(NCH):
            nc.tensor.matmul(
                scT_ps[:, c * 128 : (c + 1) * 128],
                sT_t[:, c * 128 : (c + 1) * 128],
                qkT_t[:],
                start=True,
                stop=True,
            )
        th_t = work.tile([128, N], f32, tag="th", name="th_sb")
        nc.scalar.activation(th_t[:, 0:128], scT_ps[:, 0:128], AF.Tanh)
        nc.scalar.activation(th_t[:, 128:N], scT_ps[:, 128:N], AF.Tanh)

        # xi-side trig args: [m w xi]_m  (xi = z_i@W1iT + b1)
        xit_ps = ps512.tile([128, MH], f32, tag="b512", name="xit_ps")
        nc.tensor.matmul(xit_ps[:], zTi_s, rit_t[:], start=True, stop=False)
        nc.tensor.matmul(xit_ps[:], ones_s, bit_s, start=False, stop=True)
        xit_t = work.tile([128, MH], f32, tag="xit", name="xit_sb")
        nc.vector.tensor_copy(xit_t[:], xit_ps[:])

        # z-column blocks of F via PE transpose of zT chunks
        zt_ps = []
        for c in range(NCH):
            zp = psB.tile([128, 256], f16, tag="psB", name=f"zt{c}")
            nc.tensor.transpose(zp[:, 0:128], zT_s[:, c * 128 : (c + 1) * 128], id_s)
            zt_ps.append(zp)

        # E = exp(2*scT) = (1+th)/(1-th), diag zeroed; chunk 0 first so the
        # first moment matmul can start early.
        r1_t = work.tile([128, N], f32, tag="r1", name="r1")
        r2_t = work.tile([128, N], f32, tag="r2", name="r2")
        E_t = work.tile([128, N], f16, tag="E", name="E")
        for ci, (lo, hi) in enumerate(((0, 128), (128, N))):
            nc.vector.tensor_scalar(
                r1_t[:, lo:hi], th_t[:, lo:hi], -1.0, 1.0, ALU.mult, ALU.add
            )
            nc.vector.reciprocal_approx_fast(r2_t[:, lo:hi], r1_t[:, lo:hi])
            nc.vector.scalar_tensor_tensor(
                E_t[:, lo:hi], th_t[:, lo:hi], 1.0, r2_t[:, lo:hi],
                ALU.add, ALU.mult,
            )
            # E *= (d != blk*128): zeroes the attention diagonal
            nc.vector.scalar_tensor_tensor(
                E_t[:, lo:hi], d_t[:, lo:hi], blkv_s, E_t[:, lo:hi],
                ALU.not_equal, ALU.mult,
            )
            for c in ((0, 1) if ci == 0 else (2, 3)):
                nc.vector.tensor_copy(F_t[c][:, F_Z : F_Z + H], zt_ps[c][:, 0:128])

        # j-side features + moments, pipelined per chunk
        mom_ps = psmom.tile([128, 1024], f32, tag="mom", name="mom_ps")
        slices = [(0, 512), (512, NF)]
        for c in range(NCH):
            xj_ps = ps512.tile([128, MH], f32, tag="b512", name=f"xj{c}")
            nc.tensor.matmul(
                xj_ps[:], zT_s[:, c * 128 : (c + 1) * 128], RWj_t[:],
                start=True, stop=True,
            )
            fc = F_t[c]
            nc.scalar.activation(fc[:, F_SIN : F_SIN + MH], xj_ps[:], AF.Sin)
            nc.scalar.activation(
                fc[:, F_COS : F_COS + MH], xj_ps[:], AF.Sin, bias=hpi[:, 0:1]
            )
            for s0, s1 in slices:
                nc.tensor.matmul(
                    mom_ps[:, s0:s1],
                    E_t[:, c * 128 : (c + 1) * 128],
                    fc[:, s0:s1],
                    start=(c == 0),
                    stop=(c == NCH - 1),
                )

        # xi-side trig (late in ACT queue: only needed by the combine)
        XiS = work.tile([128, MH], f16, tag="XiS", name="XiS")
        nc.scalar.activation(XiS[:], xit_t[:], AF.Sin)
        XiC = work.tile([128, MH], f16, tag="XiC", name="XiC")
        nc.scalar.activation(XiC[:], xit_t[:], AF.Sin, bias=hpi[:, 0:1])

        # combine: P = (XiS/ssum)*MCos + (XiC/ssum)*MSin; Tfin^T via
        # am-scaled-identity transpose-matmuls accumulating in PSUM.
        rs_t = work.tile([128, 1], f32, tag="rs", name="rs")
        nc.vector.reciprocal(rs_t[:], mom_ps[:, 0:1])
        Mzn_t = work.tile([128, H], f16, tag="Mzn", name="Mzn")
        nc.scalar.activation(
            Mzn_t[:], mom_ps[:, F_Z : F_Z + H], AF.Identity, scale=rs_t[:, 0:1]
        )
        P2 = work.tile([128, MH], f16, tag="P2", name="P2")
        nc.vector.scalar_tensor_tensor(
            P2[:], XiC[:], rs_t[:, 0:1], mom_ps[:, F_SIN : F_SIN + MH],
            ALU.mult, ALU.mult,
        )
        P1 = work.tile([128, MH], f16, tag="P1", name="P1")
        nc.vector.scalar_tensor_tensor(
            P1[:], XiS[:], rs_t[:, 0:1], mom_ps[:, F_COS : F_COS + MH],
            ALU.mult, ALU.mult,
        )
        P = work.tile([128, MH], f16, tag="P", name="P")
        nc.vector.tensor_tensor(P[:], P1[:], P2[:], ALU.add)

        # epilogue, all in [h, i] layout; u = (W2@..@W3).T contributions
        u_ps = psA.tile([H, IPC], f32, tag="u", name="u_ps")
        nc.tensor.matmul(u_ps[:], W2I3_s, zTi_s, start=True, stop=False)
        mT_ps = psB.tile([128, 256], f16, tag="psB", name="mT_ps")
        nc.tensor.transpose(mT_ps[:, 0:128], Mzn_t[:], id_s)
        MzT = work.tile([128, IPC], f16, tag="MzT", name="MzT")
        nc.scalar.activation(MzT[:], mT_ps[:, 0:128], AF.Copy)
        PT_ps = psq.tile([128, 256], f16, tag="qk", name="PT_ps")
        PT32 = PT_ps[:, 0:256].bitcast(f32)
        for m in range(M):
            nc.tensor.matmul(
                PT32, P[:, m * H : (m + 1) * H],
                idam_t[:, m * 128 : (m + 1) * 128],
                start=(m == 0), stop=(m == M - 1),
            )
        acc_t = work.tile([128, IPC], f16, tag="acc", name="acc")
        nc.vector.tensor_copy(acc_t[:], PT32)
        nc.tensor.matmul(u_ps[:], W2J3_s, MzT[:], start=False, stop=False)
        nc.tensor.matmul(u_ps[:], W2T3_s, acc_t[:], start=False, stop=True)

        t3_t = work.tile([H, IPC], f16, tag="t3", name="t3_sb")
        nc.scalar.activation(t3_t[:], u_ps[:], AF.Tanh, bias=b23_s)
        dz_ps = psA.tile([H, IPC], f32, tag="u", name="dz_ps")
        nc.tensor.matmul(dz_ps[:], W4T_s, t3_t[:], start=True, stop=True)
        dzT = work.tile([H, IPC], f16, tag="dzT", name="dzT_sb")
        nc.scalar.activation(dzT[:], dz_ps[:], AF.Identity, bias=b4_s)
        nc.sync.dma_start(out[:, :], dzT[:])

    nc.finalize()
    return nc


def _get_nc():
    if "nc" not in _CACHE:
        _CACHE["nc"] = _build()
    return _CACHE["nc"]


def kernel(**inputs):
    global LAST_RESULTS
    from concourse.bass_utils import run_bass_kernel_spmd

    f = np.float32
    z = np.asarray(inputs["z"], f)
    s_t = np.asarray(inputs["s_t"], f)
    W1 = np.asarray(inputs["W1"], f)
    b1 = np.asarray(inputs["b1"], f)
    W2 = np.asarray(inputs["W2"], f)
    b2 = np.asarray(inputs["b2"], f)
    Wq = np.asarray(inputs["Wq"], f)
    bq = np.asarray(inputs["bq"], f)
    Wk = np.asarray(inputs["Wk"], f)
    W3 = np.asarray(inputs["W3"], f)
    b3 = np.asarray(inputs["b3"], f)
    W4 = np.asarray(inputs["W4"], f)
    b4 = np.asarray(inputs["b4"], f)

    h16 = np.float16
    tr = lambda m: np.ascontiguousarray(m.T, f)

    rt = f(1.0 / (2.0 * np.sqrt(H)))
    W1iT = tr(W1[:, :H])
    W1jT = tr(W1[:, H:])
    W2T = tr(W2)
    W3T = tr(W3)
    QKmat = (Wq.T @ Wk) * rt
    bqk = (bq @ Wk) * rt
    brow_it = np.concatenate([(m + 1) * W * b1 for m in range(M)])
    W2T3 = W2T @ W3T
    W2J3 = (LIN_C * (W1jT @ W2T)) @ W3T
    W2I3 = (LIN_C * (W1iT @ W2T)) @ W3T
    b23 = (b2 + LIN_C * (b1 @ W2T)) @ W3T + b3

    rows = np.zeros((1, ROWS), h16)
    rows[0, R_ONES : R_ONES + 128] = 1.0
    rows[0, R_BIT : R_BIT + MH] = brow_it.astype(h16)
    rows[0, R_BQK : R_BQK + 32] = bqk.astype(h16)

    bigA_shared = np.zeros((128, BIGA), h16)
    bigA_shared[:, A_QK : A_QK + 32] = QKmat.astype(h16)
    bigA_shared[:, A_ID : A_ID + 128] = np.eye(128, dtype=h16)
    bigA_shared[:, A_W1I : A_W1I + 128] = W1iT.astype(h16)
    bigA_shared[:, A_W1J : A_W1J + 128] = W1jT.astype(h16)
    bigC = np.zeros((128, BIGC), h16)
    bigC[:, C_W2T3 : C_W2T3 + 128] = W2T3.astype(h16)
    bigC[:, C_W2J3 : C_W2J3 + 128] = W2J3.astype(h16)
    bigC[:, C_W2I3 : C_W2I3 + 128] = W2I3.astype(h16)
    bigC[:, C_W4T : C_W4T + 128] = tr(W4).astype(h16)

    in_maps = []
    for c in range(NC):
        b, blk = divmod(c, CPB)
        i0 = blk * IPC
        bigA = bigA_shared.copy()
        bigA[:, A_ZTI : A_ZTI + 128] = z[b, i0 : i0 + IPC].T.astype(h16)
        bigB = np.ascontiguousarray(z[b].T.astype(h16))
        bcols = np.zeros((H, 4), f)
        bcols[:, 0] = b23
        bcols[:, 1] = b4
        bcols[:, 2] = blk * 128
        in_maps.append(
            dict(
                bigA=bigA,
                sT=s_t[b].T.astype(h16),
                rows=rows,
                bigB=bigB,
                bigC=bigC,
                bcols=bcols,
            )
        )

    nc = _get_nc()
    res = run_bass_kernel_spmd(nc, in_maps, core_ids=list(range(NC)))
    LAST_RESULTS = res

    dz = np.empty((B, N, H), dtype=f)
    for c in range(NC):
        b, blk = divmod(c, CPB)
        i0 = blk * IPC
        dz[b, i0 : i0 + IPC, :] = res.results[c]["out"].T.astype(f)
    return dz


# revision 24
# speedup vs baseline: 1.5678x; 1.0213x over previous
"""Trainium2 Bass kernel for the ODEFunc GNN message-passing module.

Math (B=2, N=512, H=128, O=32):
    q = z @ Wq.T + bq ;  k = s_t @ Wk.T + bk
    scores = (q @ k.T)/sqrt(H), diagonal masked to -inf
    attn = softmax_j(scores)
    U    = sum_j attn[i,j] * tanh(xi_i + yj_j)      (xi = z@W1i.T + b1, yj = z@W1j.T)
    agg  = U @ W2.T + b2     (softmax rows sum to 1 -> W2 moves after aggregation)
    dz   = tanh(agg @ W3.T + b3) @ W4.T + b4

Key trick: expand tanh in a factorized basis
    tanh(x) ~ LIN_C*x + sum_m AM[m]*sin(m*W*x)        on |x| <= 4.35
so with sin(m w (xi+yj)) = sin(m w xi)cos(m w yj) + cos(m w xi)sin(m w yj),
the attention aggregation becomes moment matmuls E^T @ [1 | z | sin | cos]
with E[j,i] = exp(scores) (unnormalized, diag-zeroed).  The xi-linear and
z-moment-linear terms fold into extra epilogue matmuls; W3 is folded into
the W2-stage matrices (W2?3 = W2? @ W3T) so the epilogue is two matmul
stages; 1/ssum folds into the combine via the ones-column moment.

exp(s) = (1+tanh(s/2))/(1-tanh(s/2)) so sin+tanh suffice -> a single
manually-placed LoadActFuncSet(silu_and_others) covers every activation.
q/k projections fold into one [H,O] matrix (bk cancels in softmax).
On-chip derivations minimize input DMA: diag mask via iota+compare, the
m-scaled weight blocks via DVE scalar muls, F's z-columns via PE
transposes of zT.  All matmul operands fp16; fp32 PSUM accumulation.

Sharding: 1024 (b,i) pairs over 8 cores (batch-major, 128 i's per core).
"""

import numpy as np

B, N, H, O = 2, 512, 128, 32
NC = 8
CPB = NC // B  # cores per batch = 4
IPC = N // CPB  # i's per core = 128
NCH = N // 128  # j chunks = 4

# tanh(x) ~ LIN_C*x + sum_m AM[m] sin((m+1) W x), minimax fit on [-4.35, 4.35]
W = 0.9130
LIN_C = 0.289778
AM = [0.463016, 0.103367, 0.026572]
M = 3
MH = M * H  # 384
NF = 1 + H + 2 * MH  # 897 feature cols: [1 | sin | z | cos]
HALF_PI = 1.5707963267948966
SILU_SET_ID = 18  # silu_and_others: contains both sin and tanh

# bigA packed columns (fp16, [128, .]) -- everything the front needs
A_ZTI = 0             # zTi   [H, 128]
A_QK = 128            # QKmat [H, 32]
A_ID = 160            # identity [128, 128]
A_W1I = 288           # W1iT  [H, H]
A_W1J = 416           # W1jT  [H, H]
BIGA = 544
# bigB -- just zT
B_ZT = 0              # zT    [H, N]
BIGB = N
# F feature column layout: [1 | sin | z | cos]
F_SIN = 1
F_Z = 1 + MH
F_COS = 1 + MH + H
# bigC packed columns -- epilogue weights
C_W2T3 = 0            # W2T@W3T [H, H]
C_W2J3 = 128          # W2J@W3T
C_W2I3 = 256          # W2I@W3T
C_W4T = 384           # W4T
BIGC = 512
# rows packed (fp16, [1, .])
R_ONES = 0
R_BIT = 128           # brow_it [1, MH]
R_BQK = 128 + MH      # bqk [1, 32]
ROWS = 160 + MH

_CACHE = {}

# Stash of the last BassKernelResults (exec_time_ns etc.) for test harnesses.
LAST_RESULTS = None


def _build():
    from contextlib import ExitStack

    import concourse.tile as tile
    from concourse import bacc, mybir

    f32 = mybir.dt.float32
    f16 = mybir.dt.float16
    i32 = mybir.dt.int32
    AF = mybir.ActivationFunctionType
    ALU = mybir.AluOpType

    nc = bacc.Bacc(trn_type="TRN2")

    bigA = nc.dram_tensor("bigA", [128, BIGA], f16, kind="ExternalInput")
    sT = nc.dram_tensor("sT", [O, N], f16, kind="ExternalInput")
    rows = nc.dram_tensor("rows", [1, ROWS], f16, kind="ExternalInput")
    bigB = nc.dram_tensor("bigB", [128, BIGB], f16, kind="ExternalInput")
    bigC = nc.dram_tensor("bigC", [128, BIGC], f16, kind="ExternalInput")
    bcols = nc.dram_tensor("bcols", [H, 4], f32, kind="ExternalInput")
    out = nc.dram_tensor("out", [H, IPC], f16, kind="ExternalOutput")

    with tile.TileContext(nc) as tc, ExitStack() as ctx:
        const = ctx.enter_context(tc.tile_pool(name="const", bufs=1))
        work = ctx.enter_context(tc.tile_pool(name="work", bufs=1))
        fpool = ctx.enter_context(tc.tile_pool(name="fpool", bufs=4))
        pssc = ctx.enter_context(tc.tile_pool(name="pssc", bufs=1, space="PSUM"))
        psxi = ctx.enter_context(tc.tile_pool(name="psxi", bufs=1, space="PSUM"))
        psxj = ctx.enter_context(tc.tile_pool(name="psxj", bufs=1, space="PSUM"))
        psA = ctx.enter_context(tc.tile_pool(name="psA", bufs=1, space="PSUM"))
        psq = ctx.enter_context(tc.tile_pool(name="psq", bufs=1, space="PSUM"))
        psmom = ctx.enter_context(tc.tile_pool(name="psmom", bufs=1, space="PSUM"))

        # single activation-table load (sin + tanh live in silu_and_others)
        ld = mybir.InstLoadActFuncSet(
            act_func_set_id=SILU_SET_ID,
            name=nc.get_next_instruction_name(),
            engine=mybir.EngineType.Activation,
            ins=[],
            outs=[],
        )
        nc.scalar.add_instruction(ld)

        with tc.high_priority():
            bigA_t = const.tile([128, BIGA], f16, tag="bigA", name="bigA_sb")
            nc.sync.dma_start(bigA_t[:], bigA[:, :])
            sT_t = const.tile([O, N], f16, tag="sT", name="sT_sb")
            nc.sync.dma_start(sT_t[:], sT[:, :])
            rows_t = const.tile([1, ROWS], f16, tag="rows", name="rows_sb")
            nc.sync.dma_start(rows_t[:], rows[:, :])
            bcols_t = const.tile([H, 4], f32, tag="bcols", name="bcols_sb")
            nc.sync.dma_start(bcols_t[:], bcols[:, :])
        bigB_t = const.tile([128, BIGB], f16, tag="bigB", name="bigB_sb")
        nc.sync.dma_start(bigB_t[:], bigB[:, :])
        bigC_t = const.tile([128, BIGC], f16, tag="bigC", name="bigC_sb")
        nc.sync.dma_start(bigC_t[:], bigC[:, :])

        zTi_s = bigA_t[:, A_ZTI : A_ZTI + 128]
        QK_s = bigA_t[:, A_QK : A_QK + 32]
        W1iT_s = bigA_t[:, A_W1I : A_W1I + 128]
        id_s = bigA_t[:, A_ID : A_ID + 128]
        zT_s = bigB_t[:, B_ZT : B_ZT + N]
        W1jT_s = bigA_t[:, A_W1J : A_W1J + 128]
        W2T3_s = bigC_t[:, C_W2T3 : C_W2T3 + 128]
        W2J3_s = bigC_t[:, C_W2J3 : C_W2J3 + 128]
        W2I3_s = bigC_t[:, C_W2I3 : C_W2I3 + 128]
        W4T_s = bigC_t[:, C_W4T : C_W4T + 128]
        ones_s = rows_t[:, R_ONES : R_ONES + 128]
        bit_s = rows_t[:, R_BIT : R_BIT + MH]
        bqk_s = rows_t[:, R_BQK : R_BQK + 32]
        b23_s = bcols_t[:, 0:1]
        b4_s = bcols_t[:, 1:2]
        blkv_s = bcols_t[:, 2:3]  # blk*128, per core

        hpi = work.tile([128, 1], f32, tag="hpi", name="hpi")
        nc.vector.memset(hpi[:], HALF_PI)

        # on-chip derived weights (DVE, ready as soon as bigA lands)
        RWj_t = work.tile([128, MH], f16, tag="RWj", name="RWj_sb")
        for m in range(M):
            nc.vector.tensor_scalar_mul(
                RWj_t[:, m * H : (m + 1) * H], W1jT_s, float((m + 1) * W)
            )
        rit_t = work.tile([128, MH], f16, tag="rit", name="rit_sb")
        for m in range(M):
            nc.vector.tensor_scalar_mul(
                rit_t[:, m * H : (m + 1) * H], W1iT_s, float((m + 1) * W)
            )
        idam_t = work.tile([128, M * 128], f16, tag="idam", name="idam_sb")
        for m in range(M):
            nc.vector.tensor_scalar_mul(
                idam_t[:, m * 128 : (m + 1) * 128], id_s, float(AM[m])
            )

        # iota d[p, col] = col - p  (for the diag mask compare)
        d_t = work.tile([128, N], i32, tag="d", name="d_sb")
        nc.gpsimd.iota(d_t[:], [[1, N]], base=0, channel_multiplier=-1)

        # F feature pair-tiles [1 | sin | z | cos]; each trig ACT covers two
        # j-chunks (amortizes the fixed ACT overhead)
        F_p = []
        for p in range(2):
            fp_ = fpool.tile([128, 2, NF], f16, tag="F", name=f"F{p}")
            nc.vector.memset(fp_[:, :, 0:1], 1.0)
            F_p.append(fp_)
        F_blk = [(F_p[0], 0), (F_p[0], 1), (F_p[1], 0), (F_p[1], 1)]

        # ---- PE phase 1: xj pair 0 (feature path heads the machine) ----
        xjp0 = psxj.tile([128, 2, 512], f32, tag="xj", name="xjp0")
        for cc in range(2):
            nc.tensor.matmul(
                xjp0[:, cc, 0:MH], zT_s[:, cc * 128 : (cc + 1) * 128], RWj_t[:],
                start=True, stop=True,
            )
        # ACT 1-2: sin/cos for chunks 0,1
        nc.scalar.activation(F_p[0][:, :, F_SIN : F_SIN + MH], xjp0[:, :, 0:MH], AF.Sin)
        nc.scalar.activation(
            F_p[0][:, :, F_COS : F_COS + MH], xjp0[:, :, 0:MH], AF.Sin,
            bias=hpi[:, 0:1],
        )

        # ---- scores path ----
        qk_ps = psq.tile([32, 256], f16, tag="qk", name="qk_ps")
        qk32 = qk_ps[:, 0:256].bitcast(f32)
        nc.tensor.matmul(qk32, QK_s, zTi_s, start=True, stop=False)
        nc.tensor.matmul(qk32, bqk_s, ones_s, start=False, stop=True)
        qkT_t = work.tile([32, 128], f16, tag="qkT", name="qkT_sb")
        nc.vector.tensor_copy(qkT_t[:], qk32)

        scT_ps = pssc.tile([128, N], f32, tag="sc", name="scT_ps")
        for c in range(NCH):
            nc.tensor.matmul(
                scT_ps[:, c * 128 : (c + 1) * 128],
                sT_t[:, c * 128 : (c + 1) * 128],
                qkT_t[:],
                start=True,
                stop=True,
            )
        th_t = work.tile([128, N], f32, tag="th", name="th_sb")
        nc.scalar.activation(th_t[:, 0:128], scT_ps[:, 0:128], AF.Tanh)
        nc.scalar.activation(th_t[:, 128:N], scT_ps[:, 128:N], AF.Tanh)

        # xi-side trig args: [m w xi]_m  (xi = z_i@W1iT + b1)
        xit_ps = psxi.tile([128, 512], f32, tag="xi", name="xit_ps")
        nc.tensor.matmul(xit_ps[:, 0:MH], zTi_s, rit_t[:], start=True, stop=False)
        nc.tensor.matmul(xit_ps[:, 0:MH], ones_s, bit_s, start=False, stop=True)
        xit_t = work.tile([128, MH], f32, tag="xit", name="xit_sb")
        nc.vector.tensor_copy(xit_t[:], xit_ps[:, 0:MH])
        XiS = work.tile([128, MH], f16, tag="XiS", name="XiS")
        nc.scalar.activation(XiS[:], xit_t[:], AF.Sin)
        XiC = work.tile([128, MH], f16, tag="XiC", name="XiC")
        nc.scalar.activation(XiC[:], xit_t[:], AF.Sin, bias=hpi[:, 0:1])

        # z-column blocks of F via PE transpose of zT chunks (psq chain)
        zt_ps = []
        for c in range(NCH):
            zp = psq.tile([128, 256], f16, tag="qk", name=f"zt{c}")
            nc.tensor.transpose(zp[:, 0:128], zT_s[:, c * 128 : (c + 1) * 128], id_s)
            zt_ps.append(zp)

        # E = exp(2*scT) = (1+th)/(1-th), diag zeroed; chunk 0 split out so
        # the first moment matmuls can start early.  Fz copies interleaved.
        r1_t = work.tile([128, N], f32, tag="r1", name="r1")
        r2_t = work.tile([128, N], f32, tag="r2", name="r2")
        E_t = work.tile([128, N], f16, tag="E", name="E")
        for ci, (lo, hi) in enumerate(((0, 128), (128, N))):
            nc.vector.tensor_scalar(
                r1_t[:, lo:hi], th_t[:, lo:hi], -1.0, 1.0, ALU.mult, ALU.add
            )
            nc.vector.reciprocal_approx_fast(r2_t[:, lo:hi], r1_t[:, lo:hi])
            nc.vector.scalar_tensor_tensor(
                E_t[:, lo:hi], th_t[:, lo:hi], 1.0, r2_t[:, lo:hi],
                ALU.add, ALU.mult,
            )
            nc.vector.scalar_tensor_tensor(
                E_t[:, lo:hi], d_t[:, lo:hi], blkv_s, E_t[:, lo:hi],
                ALU.not_equal, ALU.mult,
            )
            for c in ((0, 1) if ci == 0 else (2, 3)):
                ft, cc = F_blk[c]
                nc.vector.tensor_copy(ft[:, cc, F_Z : F_Z + H], zt_ps[c][:, 0:128])

        # ---- xj pair 1 + trig ----
        xjp1 = psxj.tile([128, 2, 512], f32, tag="xj", name="xjp1")
        for cc in range(2):
            c = 2 + cc
            nc.tensor.matmul(
                xjp1[:, cc, 0:MH], zT_s[:, c * 128 : (c + 1) * 128], RWj_t[:],
                start=True, stop=True,
            )
        nc.scalar.activation(F_p[1][:, :, F_SIN : F_SIN + MH], xjp1[:, :, 0:MH], AF.Sin)
        nc.scalar.activation(
            F_p[1][:, :, F_COS : F_COS + MH], xjp1[:, :, 0:MH], AF.Sin,
            bias=hpi[:, 0:1],
        )

        # ---- moments ----
        mom_ps = psmom.tile([128, 1024], f32, tag="mom", name="mom_ps")
        slices = [(0, 512), (512, NF)]
        for c in range(NCH):
            ft, cc = F_blk[c]
            for s0, s1 in slices:
                nc.tensor.matmul(
                    mom_ps[:, s0:s1],
                    E_t[:, c * 128 : (c + 1) * 128],
                    ft[:, cc, s0:s1],
                    start=(c == 0),
                    stop=(c == NCH - 1),
                )

        # combine: P = (XiS/ssum)*MCos + (XiC/ssum)*MSin; Tfin^T via
        # am-scaled-identity transpose-matmuls accumulating in PSUM.
        rs_t = work.tile([128, 1], f32, tag="rs", name="rs")
        nc.vector.reciprocal(rs_t[:], mom_ps[:, 0:1])
        Mzn_t = work.tile([128, H], f16, tag="Mzn", name="Mzn")
        nc.scalar.activation(
            Mzn_t[:], mom_ps[:, F_Z : F_Z + H], AF.Identity, scale=rs_t[:, 0:1]
        )
        P2 = work.tile([128, MH], f16, tag="P2", name="P2")
        nc.vector.scalar_tensor_tensor(
            P2[:], XiC[:], rs_t[:, 0:1], mom_ps[:, F_SIN : F_SIN + MH],
            ALU.mult, ALU.mult,
        )
        P1 = work.tile([128, MH], f16, tag="P1", name="P1")
        nc.vector.scalar_tensor_tensor(
            P1[:], XiS[:], rs_t[:, 0:1], mom_ps[:, F_COS : F_COS + MH],
            ALU.mult, ALU.mult,
        )
        P = work.tile([128, MH], f16, tag="P", name="P")
        nc.vector.tensor_tensor(P[:], P1[:], P2[:], ALU.add)

        # epilogue, all in [h, i] layout; W3 folded into the W2-stage mats
        u_ps = psA.tile([H, IPC], f32, tag="u", name="u_ps")
        nc.tensor.matmul(u_ps[:], W2I3_s, zTi_s, start=True, stop=False)
        mT_ps = psq.tile([128, 256], f16, tag="qk", name="mT_ps")
        nc.tensor.transpose(mT_ps[:, 0:128], Mzn_t[:], id_s)
        MzT = work.tile([128, IPC], f16, tag="MzT", name="MzT")
        nc.scalar.activation(MzT[:], mT_ps[:, 0:128], AF.Copy)
        PT_ps = psq.tile([128, 256], f16, tag="qk", name="PT_ps")
        PT32 = PT_ps[:, 0:256].bitcast(f32)
        for m in range(M):
            nc.tensor.matmul(
                PT32, P[:, m * H : (m + 1) * H],
                idam_t[:, m * 128 : (m + 1) * 128],
                start=(m == 0), stop=(m == M - 1),
            )
        acc_t = work.tile([128, IPC], f16, tag="acc", name="acc")
        nc.vector.tensor_copy(acc_t[:], PT32)
        nc.tensor.matmul(u_ps[:], W2J3_s, MzT[:], start=False, stop=False)
        nc.tensor.matmul(u_ps[:], W2T3_s, acc_t[:], start=False, stop=True)

        t3_t = work.tile([H, IPC], f16, tag="t3", name="t3_sb")
        nc.scalar.activation(t3_t[:], u_ps[:], AF.Tanh, bias=b23_s)
        dz_ps = psA.tile([H, IPC], f32, tag="u", name="dz_ps")
        nc.tensor.matmul(dz_ps[:], W4T_s, t3_t[:], start=True, stop=True)
        dzT = work.tile([H, IPC], f16, tag="dzT", name="dzT_sb")
        nc.scalar.activation(dzT[:], dz_ps[:], AF.Identity, bias=b4_s)
        nc.sync.dma_start(out[:, :], dzT[:])

    nc.finalize()
    return nc


def _get_nc():
    if "nc" not in _CACHE:
        _CACHE["nc"] = _build()
    return _CACHE["nc"]


def kernel(**inputs):
    global LAST_RESULTS
    from concourse.bass_utils import run_bass_kernel_spmd

    f = np.float32
    z = np.asarray(inputs["z"], f)
    s_t = np.asarray(inputs["s_t"], f)
    W1 = np.asarray(inputs["W1"], f)
    b1 = np.asarray(inputs["b1"], f)
    W2 = np.asarray(inputs["W2"], f)
    b2 = np.asarray(inputs["b2"], f)
    Wq = np.asarray(inputs["Wq"], f)
    bq = np.asarray(inputs["bq"], f)
    Wk = np.asarray(inputs["Wk"], f)
    W3 = np.asarray(inputs["W3"], f)
    b3 = np.asarray(inputs["b3"], f)
    W4 = np.asarray(inputs["W4"], f)
    b4 = np.asarray(inputs["b4"], f)

    h16 = np.float16
    tr = lambda m: np.ascontiguousarray(m.T, f)

    rt = f(1.0 / (2.0 * np.sqrt(H)))
    W1iT = tr(W1[:, :H])
    W1jT = tr(W1[:, H:])
    W2T = tr(W2)
    W3T = tr(W3)
    QKmat = (Wq.T @ Wk) * rt
    bqk = (bq @ Wk) * rt
    brow_it = np.concatenate([(m + 1) * W * b1 for m in range(M)])
    W2T3 = W2T @ W3T
    W2J3 = (LIN_C * (W1jT @ W2T)) @ W3T
    W2I3 = (LIN_C * (W1iT @ W2T)) @ W3T
    b23 = (b2 + LIN_C * (b1 @ W2T)) @ W3T + b3

    rows = np.zeros((1, ROWS), h16)
    rows[0, R_ONES : R_ONES + 128] = 1.0
    rows[0, R_BIT : R_BIT + MH] = brow_it.astype(h16)
    rows[0, R_BQK : R_BQK + 32] = bqk.astype(h16)

    bigA_shared = np.zeros((128, BIGA), h16)
    bigA_shared[:, A_QK : A_QK + 32] = QKmat.astype(h16)
    bigA_shared[:, A_ID : A_ID + 128] = np.eye(128, dtype=h16)
    bigA_shared[:, A_W1I : A_W1I + 128] = W1iT.astype(h16)
    bigA_shared[:, A_W1J : A_W1J + 128] = W1jT.astype(h16)
    bigC = np.zeros((128, BIGC), h16)
    bigC[:, C_W2T3 : C_W2T3 + 128] = W2T3.astype(h16)
    bigC[:, C_W2J3 : C_W2J3 + 128] = W2J3.astype(h16)
    bigC[:, C_W2I3 : C_W2I3 + 128] = W2I3.astype(h16)
    bigC[:, C_W4T : C_W4T + 128] = tr(W4).astype(h16)

    in_maps = []
    for c in range(NC):
        b, blk = divmod(c, CPB)
        i0 = blk * IPC
        bigA = bigA_shared.copy()
        bigA[:, A_ZTI : A_ZTI + 128] = z[b, i0 : i0 + IPC].T.astype(h16)
        bigB = np.ascontiguousarray(z[b].T.astype(h16))
        bcols = np.zeros((H, 4), f)
        bcols[:, 0] = b23
        bcols[:, 1] = b4
        bcols[:, 2] = blk * 128
        in_maps.append(
            dict(
                bigA=bigA,
                sT=s_t[b].T.astype(h16),
                rows=rows,
                bigB=bigB,
                bigC=bigC,
                bcols=bcols,
            )
        )

    nc = _get_nc()
    res = run_bass_kernel_spmd(nc, in_maps, core_ids=list(range(NC)))
    LAST_RESULTS = res

    dz = np.empty((B, N, H), dtype=f)
    for c in range(NC):
        b, blk = divmod(c, CPB)
        i0 = blk * IPC
        dz[b, i0 : i0 + IPC, :] = res.results[c]["out"].T.astype(f)
    return dz
